# revision 1
# baseline (speedup 1.0000x reference)
"""GNN message-passing (ArtemisNet) distributed Bass kernel for 8 TRN2 cores, v2.

Strategy (v2 — descriptor-exact gathers + chunked AllGather):
- dst-sharding: core c owns nodes [c*NSH, (c+1)*NSH). Edges assigned by dst.
- Gather of source-node rows via dma_gather (int16 idx, X/Y table halves,
  chunk-major table layout). Per-(window,half) gather calls with runtime
  exact counts (num_idxs_reg via value_load): pad slots emit NO descriptors.
- Segment aggregation on TensorEngine: per 128-dst window, PSUM accumulates
  G_sub^T @ onehot_sub; pad slots have dstrel=-1 -> zero one-hot columns.
- Node-wise GEMMs feature-major; BN+ReLU folded into one ACT op.
- h tables republished per hop via per-chunk AllGathers (7 chunks/hop), so
  the next hop's gathers wait only on a small tail chunk.
"""

import dataclasses
import numpy as np
import ml_dtypes

import concourse.bass as bass
import concourse.bacc as bacc
import concourse.tile as tile
import concourse.mybir as mybir

BF16 = mybir.dt.bfloat16
F32 = mybir.dt.float32
I16 = mybir.dt.int16
I32 = mybir.dt.int32
AF = mybir.ActivationFunctionType
ALU = mybir.AluOpType


@dataclasses.dataclass
class Cfg:
    N: int = 50000
    E: int = 800000
    NC: int = 8
    D_IN: int = 64
    D_E: int = 32
    H: int = 128
    EPS: float = 1e-5
    NSH: int = 6250          # nodes per core
    WSZ: int = 128           # dst window size
    NW: int = 50             # windows per core (NW*WSZ >= NSH)
    SHPAD: int = 6272        # padded shard rows in gather table (mult of 128)

    @property
    def NPAD(self):
        return self.NW * self.WSZ

    @property
    def NWR(self):
        return self.SHPAD // 128

    @property
    def XROWS(self):
        return (self.SHPAD // 128 * 31 // 49) * 128 if self.SHPAD > 256 else self.SHPAD // 2

    @property
    def YROWS(self):
        return self.SHPAD - self.XROWS

    @property
    def XW(self):
        return self.XROWS // 128

    @property
    def chunks(self):
        """Window-range chunks: X chunks then Y chunks, groups of <=8."""
        cx = [(a, min(a + 8, self.XW)) for a in range(0, self.XW, 8)]
        cy = [(a, min(a + 8, self.NWR)) for a in range(self.XW, self.NWR, 8)]
        return cx, cy

    def chunk_bases(self):
        """Per-chunk row bases within the X (resp Y) tables (full, x8 ranks)."""
        cx, cy = self.chunks
        bx, acc = [], 0
        for a, b in cx:
            bx.append(acc)
            acc += self.NC * (b - a) * 128
        by, acc = [], 0
        for a, b in cy:
            by.append(acc)
            acc += self.NC * (b - a) * 128
        return bx, by

    def tblrow(self, s):
        """Vectorized: global node id -> (half(0=X,1=Y), table row)."""
        s = np.asarray(s, np.int64)
        sh = s // self.NSH
        r = s % self.NSH
        half = (r >= self.XROWS).astype(np.int64)
        row = np.where(half == 0, sh * self.XROWS + r,
                       sh * self.YROWS + (r - self.XROWS))
        return half, row


FULL = Cfg()
MINI = Cfg(N=2048, E=8192, NSH=256, NW=2, SHPAD=256)


def _wrap_idx16(a):
    """[n] int -> [128, n//16] int16 (idx i at partition i%16, col i//16; tiled x8)."""
    n = a.shape[0]
    assert n % 16 == 0
    w = a.reshape(n // 16, 16).T.astype(np.int16)
    return np.tile(w, (8, 1)).copy()


def _slotmaj(a):
    """[TOT] -> [128, TOT//128] slot i at [i%128, i//128]."""
    t = a.shape[0]
    return np.ascontiguousarray(a.reshape(t // 128, 128).T)


def hop_counts(cfg: Cfg, src, dst, c):
    sel = (dst >= c * cfg.NSH) & (dst < (c + 1) * cfg.NSH)
    s = src[sel].astype(np.int64)
    d = (dst[sel] - c * cfg.NSH).astype(np.int64)
    half, _ = cfg.tblrow(s)
    win = d // cfg.WSZ
    key = half * cfg.NW + win
    return np.bincount(key, minlength=2 * cfg.NW)


def compute_sched(cfg: Cfg, eidx):
    """Per-hop per-window sub-chunk counts (max over cores), SPMD-static."""
    sched = []
    for k in range(3):
        mx = np.zeros(2 * cfg.NW, np.int64)
        for c in range(cfg.NC):
            mx = np.maximum(mx, hop_counts(cfg, eidx[k][0], eidx[k][1], c))
        subsA = np.maximum(1, -(-mx[:cfg.NW] // 128))
        subsB = -(-mx[cfg.NW:] // 128)
        sched.append((subsA.astype(int), subsB.astype(int)))
    return sched


def sched_layout(cfg: Cfg, sub):
    """Slot bases per (half, window) from a hop schedule."""
    subsA, subsB = sub
    slotsA, slotsB = subsA * 128, subsB * 128
    a_tot = int(slotsA.sum())
    baseA = np.concatenate([[0], np.cumsum(slotsA)[:-1]])
    baseB = a_tot + np.concatenate([[0], np.cumsum(slotsB)[:-1]])
    tot = a_tot + int(slotsB.sum())
    return baseA, baseB, slotsA, slotsB, a_tot, tot


def prep_core_hop(cfg: Cfg, sub, src, dst, c, edge_attr=None):
    """Slot assignment for one (core, hop) under schedule `sub`.

    Pads are trailing within each (window, half) segment: gidx=-1, dstrel=-1.
    Returns per-(window,half) valid counts for runtime-exact gathers.
    """
    baseA, baseB, slotsA, slotsB, a_tot, tot = sched_layout(cfg, sub)
    sel = (dst >= c * cfg.NSH) & (dst < (c + 1) * cfg.NSH)
    s = src[sel].astype(np.int64)
    d = (dst[sel] - c * cfg.NSH).astype(np.int64)
    half, tblrow = cfg.tblrow(s)
    win = d // cfg.WSZ

    gidx = np.full(tot, -1, np.int64)
    dstrel = np.full(tot, -1.0, np.float32)
    ea = None
    if edge_attr is not None:
        ea = np.zeros((tot, cfg.D_E), np.float32)
        eav = edge_attr[sel]

    order = np.lexsort((d, win, half))
    d_, t_, h_, w_ = (x[order] for x in (d, tblrow, half, win))
    if edge_attr is not None:
        eav = eav[order]

    deg = np.bincount(d, minlength=cfg.NPAD).astype(np.float32)
    keys = h_ * cfg.NW + w_
    bnd = np.searchsorted(keys, np.arange(2 * cfg.NW + 1))
    cnts = (bnd[1:] - bnd[:-1]).astype(np.int64)
    slots_per = np.concatenate([slotsA, slotsB])
    assert (cnts <= slots_per).all(), f"slot overflow core {c}"
    seg_base = np.concatenate([baseA, baseB])
    pos = (seg_base[keys] + np.arange(len(keys)) - bnd[keys]).astype(np.int64)
    gidx[pos] = t_
    dstrel[pos] = (d_ - w_ * cfg.WSZ).astype(np.float32)
    if edge_attr is not None:
        ea[pos] = eav

    # zero-count segments with scheduled slots: keep one valid dummy desc
    # (row 0, dstrel=-1) so num_idxs_reg >= 1 everywhere.
    for seg in range(2 * cfg.NW):
        if cnts[seg] == 0 and slots_per[seg] > 0:
            gidx[seg_base[seg]] = 0
            cnts[seg] = 1

    cnt128 = np.zeros(128, np.int32)
    cnt128[:cfg.NW] = cnts[:cfg.NW]          # A counts
    cnt128[64:64 + cfg.NW] = cnts[cfg.NW:]   # B counts
    invdeg = (1.0 / np.maximum(deg, 1.0)).astype(np.float32)
    out = {
        "gidx": _wrap_idx16(gidx),
        "cnt": cnt128.reshape(1, 128),
        "inv": np.broadcast_to(invdeg.astype(ml_dtypes.bfloat16), (128, cfg.NPAD)).copy(),
        "dstrel": _slotmaj(dstrel.astype(ml_dtypes.bfloat16)),
    }
    if edge_attr is not None:
        out["ea"] = np.ascontiguousarray(
            ea.astype(ml_dtypes.bfloat16).reshape(tot // 128, 128, cfg.D_E))
    return out


def prep_inputs(cfg: Cfg, inp):
    """Full-host preprocessing: returns in_maps (list of dicts, one per core)."""
    x = np.asarray(inp["x"], np.float32)
    H, D_IN, D_E = cfg.H, cfg.D_IN, cfg.D_E

    # gather tables for hop 0 (chunk-major X/Y layout): bf16, x in cols 0:64
    xtX = np.zeros((cfg.NC * cfg.XROWS, 128), np.float32)
    xtY = np.zeros((cfg.NC * cfg.YROWS, 128), np.float32)
    node = np.arange(cfg.N, dtype=np.int64)
    half, row = cfg.tblrow(node)
    mX = half == 0
    xtX[row[mX], :D_IN] = x[node[mX]]
    xtY[row[mX == False], :D_IN] = x[node[~mX]]  # noqa: E712
    xtX[:, D_IN + D_E] = 1.0   # bias marker col
    xtY[:, D_IN + D_E] = 1.0
    xtX = xtX.astype(ml_dtypes.bfloat16)
    xtY = xtY.astype(ml_dtypes.bfloat16)

    W1 = np.asarray(inp["W1"], np.float32)  # [H, D_IN+D_E]
    w1c = np.zeros((D_IN + D_E + 1, H), np.float32)
    w1c[:D_IN] = W1[:, :D_IN].T
    w1c[D_IN:D_IN + D_E] = W1[:, D_IN:].T
    w1c[D_IN + D_E] = np.asarray(inp["b1"], np.float32)
    w1c = w1c.astype(ml_dtypes.bfloat16)

    def bn_fold(g, be, m, v, blin=None):
        g, be, m, v = (np.asarray(inp[k], np.float32) for k in (g, be, m, v))
        gam = g / np.sqrt(v + cfg.EPS)
        bet = be - m * gam
        if blin is not None:
            bet = bet + gam * np.asarray(inp[blin], np.float32)
        return gam.reshape(-1, 1), bet.reshape(-1, 1)

    sc1, bs1 = bn_fold("g1", "be1", "m1", "v1")
    sc2, bs2 = bn_fold("g2", "be2", "m2", "v2", "bl2")
    sc3, bs3 = bn_fold("g3", "be3", "m3", "v3", "bl3")

    W4 = np.asarray(inp["W4"], np.float32)  # [64, H+D_IN]
    w4h = W4[:, :H].T.astype(ml_dtypes.bfloat16)         # [H, 64]
    w4x = W4[:, H:].T.astype(ml_dtypes.bfloat16)         # [D_IN, 64]
    b4 = np.asarray(inp["b4"], np.float32).reshape(-1, 1)
    w5 = np.asarray(inp["W5"], np.float32).T             # [64, 1]
    b5 = np.asarray(inp["b5"], np.float32).reshape(1, 1)

    iota = np.broadcast_to(np.arange(128, dtype=np.float32), (128, 128)
                           ).astype(ml_dtypes.bfloat16)
    ident_bf = np.eye(128, dtype=ml_dtypes.bfloat16)

    shared = {
        "xtX": xtX, "xtY": xtY,
        "w1c": w1c,
        "w2l": np.asarray(inp["Wl2"], np.float32).T.astype(ml_dtypes.bfloat16),
        "w2r": np.asarray(inp["Wr2"], np.float32).T.astype(ml_dtypes.bfloat16),
        "w3l": np.asarray(inp["Wl3"], np.float32).T.astype(ml_dtypes.bfloat16),
        "w3r": np.asarray(inp["Wr3"], np.float32).T.astype(ml_dtypes.bfloat16),
        "sc1": sc1, "bs1": bs1, "sc2": sc2, "bs2": bs2, "sc3": sc3, "bs3": bs3,
        "w4h": w4h, "w4x": w4x, "b4": b4, "w5": w5, "b5": b5,
        "iota": iota, "ident_bf": ident_bf,
    }

    eidx = [np.asarray(inp[f"edge_index_{k}"]) for k in range(3)]
    ea0 = np.asarray(inp["edge_attr_0"], np.float32)
    sched = compute_sched(cfg, eidx)

    in_maps = []
    for c in range(cfg.NC):
        m = dict(shared)
        # x_ownT bf16 [D_IN, NPAD]
        xo = np.zeros((cfg.NPAD, D_IN), np.float32)
        lo, hi = c * cfg.NSH, min((c + 1) * cfg.NSH, cfg.N)
        xo[:hi - lo] = x[lo:hi]
        m["x_ownT"] = xo.T.astype(ml_dtypes.bfloat16).copy()
        for k in range(3):
            p = prep_core_hop(cfg, sched[k], eidx[k][0], eidx[k][1], c,
                              ea0 if k == 0 else None)
            m[f"gidx{k}"] = p["gidx"]
            m[f"cnt{k}"] = p["cnt"]
            m[f"dstrel{k}"] = p["dstrel"]
            m[f"inv{k}"] = p["inv"]
            if k == 0:
                m["ea"] = p["ea"]
        in_maps.append(m)
    return in_maps, sched


def build_kernel(cfg: Cfg, sched, queue_map=None):
    gather_insts = {}  # inst name -> key

    nc = bacc.Bacc("TRN2", target_bir_lowering=False, debug=False,
                   num_devices=cfg.NC, num_swdge_queues=4,
                   dynamic_dma_scratch_size=16384)
    H, D_IN, D_E = cfg.H, cfg.D_IN, cfg.D_E
    DXE = D_IN + D_E
    NW = cfg.NW
    WPB = 2 if NW % 2 == 0 else 1
    NBLK = NW // WPB
    lay = [sched_layout(cfg, sub) for sub in sched]
    MAXSUBA = max(int(sub[0].max()) for sub in sched)
    MAXSUBB = max(max(int(sub[1].max()), 1) for sub in sched)
    CX, CY = cfg.chunks
    BX, BY = cfg.chunk_bases()

    P = {}

    def par(name, shape, dt=F32, out=False):
        P[name] = nc.declare_dram_parameter(name, list(shape), dt, isOutput=out)
        return P[name]

    par("xtX", (cfg.NC * cfg.XROWS, 128), BF16)
    par("xtY", (cfg.NC * cfg.YROWS, 128), BF16)
    par("x_ownT", (D_IN, cfg.NPAD), BF16)
    par("ea", (lay[0][5] // 128, 128, D_E), BF16)
    for k in range(3):
        par(f"gidx{k}", (128, lay[k][5] // 16), I16)
        par(f"cnt{k}", (1, 128), I32)
        par(f"dstrel{k}", (128, lay[k][5] // 128), BF16)
        par(f"inv{k}", (128, cfg.NPAD), BF16)
    par("w1c", (DXE + 1, H), BF16)
    par("w2l", (H, H), BF16); par("w2r", (H, H), BF16)
    par("w3l", (H, H), BF16); par("w3r", (H, H), BF16)
    for nm in ("sc1", "bs1", "sc2", "bs2", "sc3", "bs3"):
        par(nm, (H, 1))
    par("w4h", (H, 64), BF16); par("w4x", (D_IN, 64), BF16)
    par("b4", (64, 1)); par("w5", (64, 1)); par("b5", (1, 1))
    par("iota", (128, 128), BF16)
    par("ident_bf", (128, 128), BF16)
    out_ext = par("out", (1, cfg.NPAD), out=True)

    with tile.TileContext(nc) as tc:
        with (
            tc.tile_pool(name="const", bufs=1) as cp,
            tc.tile_pool(name="invp", bufs=2) as invp,
            tc.tile_pool(name="cnp", bufs=3) as cnp,
            tc.tile_pool(name="ohp", bufs=4) as ohp,
            tc.tile_pool(name="ip", bufs=4) as ip,
            tc.tile_pool(name="hp", bufs=2) as hp,
            tc.tile_pool(name="nmp", bufs=6) as nmp,
            tc.tile_pool(name="pse", bufs=3, space="PSUM") as pse,
            tc.tile_pool(name="psn", bufs=5, space="PSUM") as psn,
            tc.tile_pool(name="dram", bufs=1, space="DRAM") as dp,
        ):
            def ld(name, dt=F32):
                t = cp.tile(list(P[name].shape), dt, tag=name)
                nc.scalar.dma_start(t[:], P[name].ap())
                return t

            w1c = ld("w1c", BF16)
            w2l = ld("w2l", BF16); w3l = ld("w3l", BF16)
            w2r = ld("w2r", BF16); w3r = ld("w3r", BF16)
            sc = [ld(f"sc{k}") for k in (1, 2, 3)]
            bs = [ld(f"bs{k}") for k in (1, 2, 3)]
            w4h = ld("w4h", BF16); w4x = ld("w4x", BF16)
            b4 = ld("b4"); w5 = ld("w5"); b5 = ld("b5")
            iota = ld("iota", BF16)
            ident_bf = ld("ident_bf", BF16)
            x_ownT = ld("x_ownT", BF16)
            iota_big = cp.tile([128, WPB * MAXSUBA, 128], BF16, tag="iota_big")
            nc.vector.tensor_copy(
                iota_big[:],
                iota[:].rearrange("p (o f) -> p o f", o=1).broadcast_to(
                    [128, WPB * MAXSUBA, 128]))

            # persistent gather ring buffers: pad slots skip their DMA and
            # must read as finite values for the masked matmul, so the rings
            # are zeroed once here and then only ever overwritten by gathers.
            RING_A = 6
            RING_B = 6
            MA = WPB * MAXSUBA
            MB = WPB * MAXSUBB
            gbufA = cp.tile([128, RING_A * MA, 128], BF16, tag="gbufA")
            gbufB = cp.tile([128, RING_B * MB, 128], BF16, tag="gbufB")
            for rp in range(RING_A):
                nc.vector.memset(gbufA[:, rp * MA:(rp + 1) * MA, :], 0.0)
            for rp in range(RING_B):
                nc.vector.memset(gbufB[:, rp * MB:(rp + 1) * MB, :], 0.0)

            htblX = [dp.tile([cfg.NC * cfg.XROWS, 128], BF16,
                             name=f"htblX{k}", tag=f"htblX{k}",
                             addr_space="Shared") for k in range(2)]
            htblY = [dp.tile([cfg.NC * cfg.YROWS, 128], BF16,
                             name=f"htblY{k}", tag=f"htblY{k}",
                             addr_space="Shared") for k in range(2)]
            bounceX = [dp.tile([cfg.XROWS, 128], BF16, name=f"bounceX{k}",
                               tag=f"bounceX{k}") for k in range(2)]
            bounceY = [dp.tile([cfg.YROWS, 128], BF16, name=f"bounceY{k}",
                               tag=f"bounceY{k}") for k in range(2)]

            # tiny warm-up collective: absorbs first-call AG overhead
            wub = dp.tile([128, 128], BF16, tag="wub")
            wuo = dp.tile([cfg.NC * 128, 128], BF16, tag="wuo",
                          addr_space="Shared")
            wz = cp.tile([128, 128], BF16, tag="wz")
            nc.vector.memset(wz[:], 0.0)
            nc.sync.dma_start(wub[:], wz[:])
            nc.gpsimd.collective_compute(
                "AllGather", ALU.bypass,
                replica_groups=[list(range(cfg.NC))],
                ins=[wub.opt()], outs=[wuo.opt()])

            h_prev = None
            h_cur = None
            cnt_regs = [nc.gpsimd.alloc_register(f"cntreg{i}")
                        for i in range(12)]
            creg_ctr = [0]

            def load_cnt(ap):
                r = cnt_regs[creg_ctr[0] % 12]
                creg_ctr[0] += 1
                nc.gpsimd.reg_load(r, ap)
                return r

            def reg_gather(inst, key):
                gather_insts[inst.ins.name] = key

            for k in range(3):
                baseA, baseB, slotsA, slotsB, a_tot, tot = lay[k]
                subsA, subsB = sched[k]
                if k == 0:
                    tblX = P["xtX"].ap()
                    tblY = P["xtY"].ap()
                else:
                    tblX = htblX[k - 1][:]
                    tblY = htblY[k - 1][:]
                inv = invp.tile([128, cfg.NPAD], BF16, tag="inv_rep",
                                name=f"invt{k}")
                nc.scalar.dma_start(inv[:], P[f"inv{k}"].ap())
                cntT = cnp.tile([1, 128], I32, tag="cntT", name=f"cntT{k}")
                nc.sync.dma_start(cntT[:], P[f"cnt{k}"].ap())

                h_prev = h_cur
                h_cur = hp.tile([128, cfg.NPAD], BF16, tag="h", name=f"h{k}")
                if k < 2:
                    bsb = cp.tile([128, cfg.NWR, 128], BF16,
                                  tag="bsb", name=f"bsb{k}")
                fdim = DXE + 1 if k == 0 else 128
                PRO = 4  # gather lookahead depth (blocks)
                state = {}

                def emit_a(j, k=k, state=state, tblX=tblX,
                           subsA=subsA, subsB=subsB, baseA=baseA, baseB=baseB,
                           cntT=cntT):
                    ws = list(range(j * WPB, (j + 1) * WPB))
                    nsa = [int(subsA[w]) for w in ws]
                    nsb = [int(subsB[w]) for w in ws]
                    nA = 128 * sum(nsa)
                    nB = 128 * sum(nsb)
                    sA0 = int(baseA[ws[0]])
                    sB0 = int(baseB[ws[0]])
                    rp = (k * NBLK + j) % RING_A
                    ga = gbufA[:, rp * MA:rp * MA + sum(nsa), :]
                    gia = ip.tile([128, nA // 16], I16, tag="gia",
                                  name=f"gia{k}_{j}")
                    nc.sync.dma_start(
                        gia[:], P[f"gidx{k}"].ap()[:, sA0 // 16:
                                                   (sA0 + nA) // 16])
                    for wl in range(WPB):
                        w = ws[wl]
                        offa = sum(nsa[:wl])
                        na_w = nsa[wl] * 128
                        cva = load_cnt(cntT[0:1, w:w + 1])
                        keyA = (k, j, wl, 0)
                        reg_gather(nc.gpsimd.dma_gather(
                            ga[:, offa:offa + nsa[wl], :], tblX,
                            gia[:, (offa * 128) // 16:
                                (offa * 128 + na_w) // 16],
                            na_w, cva, 128, single_packet=False,
                            queue_num=(queue_map or {}).get(keyA, 0)), keyA)
                    state[j] = (ga, None, nsa, nsb, nA, nB, sA0, sB0)

                def emit_b(j, k=k, state=state, tblY=tblY,
                           subsB=subsB, baseB=baseB, cntT=cntT):
                    ga, _, nsa, nsb, nA, nB, sA0, sB0 = state[j]
                    ws = list(range(j * WPB, (j + 1) * WPB))
                    if nB == 0:
                        return
                    rp = (k * NBLK + j) % RING_B
                    gb = gbufB[:, rp * MB:rp * MB + max(sum(nsb), 1), :]
                    gib = ip.tile([128, nB // 16], I16, tag="gib",
                                  name=f"gib{k}_{j}")
                    nc.sync.dma_start(
                        gib[:], P[f"gidx{k}"].ap()[:, sB0 // 16:
                                                   (sB0 + nB) // 16])
                    for wl in range(WPB):
                        w = ws[wl]
                        if nsb[wl] > 0:
                            offb = sum(nsb[:wl])
                            nb_w = nsb[wl] * 128
                            cvb = load_cnt(cntT[0:1, 64 + w:64 + w + 1])
                            keyB = (k, j, wl, 1)
                            reg_gather(nc.gpsimd.dma_gather(
                                gb[:, offb:offb + nsb[wl], :], tblY,
                                gib[:, (offb * 128) // 16:
                                    (offb * 128 + nb_w) // 16],
                                nb_w, cvb, 128, single_packet=False,
                                queue_num=(queue_map or {}).get(keyB, 0)), keyB)
                    state[j] = (ga, gb, nsa, nsb, nA, nB, sA0, sB0)

                def emit_rest(j, k=k, state=state, inv=inv, h_cur=h_cur,
                              h_prev=h_prev, fdim=fdim,
                              bsb=(bsb if k < 2 else None)):
                    ga, gb, nsa, nsb, nA, nB, sA0, sB0 = state.pop(j)
                    ws = list(range(j * WPB, (j + 1) * WPB))
                    eat_a = eat_b = None
                    if k == 0:
                        eat_a = ip.tile([128, sum(nsa), D_E], BF16,
                                        tag="eat_a", name=f"ea_a{j}")
                        nc.sync.dma_start(
                            eat_a[:],
                            P["ea"].ap()[sA0 // 128: (sA0 + nA) // 128]
                            .rearrange("r p e -> p r e"))
                        if nB > 0:
                            eat_b = ip.tile([128, sum(nsb), D_E], BF16,
                                            tag="eat_b", name=f"ea_b{j}")
                            nc.sync.dma_start(
                                eat_b[:],
                                P["ea"].ap()[sB0 // 128: (sB0 + nB) // 128]
                                .rearrange("r p e -> p r e"))
                    oa = ohp.tile([128, sum(nsa), 128], BF16, tag="oa",
                                  name=f"oa{k}_{j}")
                    dra = ip.tile([128, sum(nsa)], BF16, tag="dra",
                                  name=f"dra{k}_{j}")
                    nc.sync.dma_start(
                        dra[:], P[f"dstrel{k}"].ap()[:, sA0 // 128:
                                                     (sA0 + nA) // 128])
                    nc.vector.tensor_tensor(
                        oa[:], iota_big[:, 0:sum(nsa), :],
                        dra[:].rearrange("p (s o) -> p s o", o=1).broadcast_to(
                            [128, sum(nsa), 128]),
                        ALU.is_equal)
                    ob = None
                    if nB > 0:
                        ob = ohp.tile([128, sum(nsb), 128], BF16, tag="ob",
                                      name=f"ob{k}_{j}")
                        drb = ip.tile([128, sum(nsb)], BF16, tag="drb",
                                      name=f"drb{k}_{j}")
                        nc.sync.dma_start(
                            drb[:], P[f"dstrel{k}"].ap()[:, sB0 // 128:
                                                         (sB0 + nB) // 128])
                        nc.vector.tensor_tensor(
                            ob[:], iota_big[:, 0:sum(nsb), :],
                            drb[:].rearrange(
                                "p (s o) -> p s o", o=1).broadcast_to(
                                [128, sum(nsb), 128]),
                            ALU.is_equal)

                    for wl in range(WPB):
                        w = ws[wl]
                        offa = sum(nsa[:wl])
                        offb = sum(nsb[:wl])
                        cols = slice(w * 128, (w + 1) * 128)
                        ps = pse.tile([128, 128], F32, tag="ps",
                                      name=f"ps{k}_{w}")
                        for t in range(nsa[wl]):
                            nc.tensor.matmul(
                                ps[0:fdim, :], ga[:, offa + t, 0:fdim],
                                oa[:, offa + t, :],
                                start=(t == 0), stop=False)
                        if k == 0:
                            for t in range(nsa[wl]):
                                nc.tensor.matmul(
                                    ps[D_IN:DXE, :], eat_a[:, offa + t, :],
                                    oa[:, offa + t, :],
                                    start=False, stop=False,
                                    skip_group_check=True)
                        for t in range(nsb[wl]):
                            nc.tensor.matmul(
                                ps[0:fdim, :], gb[:, offb + t, 0:fdim],
                                ob[:, offb + t, :],
                                start=False, stop=(t == nsb[wl] - 1))
                        if k == 0:
                            for t in range(nsb[wl]):
                                nc.tensor.matmul(
                                    ps[D_IN:DXE, :], eat_b[:, offb + t, :],
                                    ob[:, offb + t, :],
                                    start=False, stop=(t == nsb[wl] - 1),
                                    skip_group_check=True)
                        rhs = nmp.tile([128, 128], BF16, tag="rhs",
                                       name=f"rhs{k}_{w}")
                        nc.vector.tensor_tensor(rhs[0:fdim, :], ps[0:fdim, :],
                                                inv[0:fdim, cols], ALU.mult)
                        ps2 = psn.tile([128, 128], F32, tag="psn",
                                       name=f"ps2{k}_{w}")
                        if k == 0:
                            nc.tensor.matmul(ps2[:], w1c[:], rhs[0:fdim, :],
                                             start=True, stop=True)
                            tmp = nmp.tile([128, 128], F32, tag="tmp",
                                           name=f"tmp{w}")
                            nc.scalar.activation(tmp[:], ps2[:], AF.Relu)
                            nc.scalar.activation(h_cur[:, cols], tmp[:],
                                                 AF.Relu, bias=bs[0][:],
                                                 scale=sc[0][:])
                        else:
                            wl_ = w2l if k == 1 else w3l
                            wr_ = w2r if k == 1 else w3r
                            nc.tensor.matmul(ps2[:], wl_[:], rhs[:],
                                             start=True, stop=False)
                            nc.tensor.matmul(ps2[:], wr_[:], h_prev[:, cols],
                                             start=False, stop=True)
                            nc.scalar.activation(h_cur[:, cols], ps2[:],
                                                 AF.Relu, bias=bs[k][:],
                                                 scale=sc[k][:])
                        if k < 2 and w < cfg.NWR:
                            pstr = psn.tile([128, 128], BF16, tag="psn",
                                            name=f"pstr{k}_{w}")
                            nc.tensor.transpose(pstr[:], h_cur[:, cols],
                                                ident_bf[:])
                            nc.scalar.activation(bsb[:, w, :], pstr[:],
                                                 AF.Copy)
                            XW = cfg.XW
                            NWR = cfg.NWR
                            if w < XW and (w % 8 == 7 or w == XW - 1):
                                w0 = (w // 8) * 8
                                nc.sync.dma_start(
                                    bounceX[k][:].rearrange(
                                        "(t p) f -> p t f",
                                        p=128)[:, w0:w + 1, :],
                                    bsb[:, w0:w + 1, :])
                                if w == XW - 1:
                                    nc.gpsimd.collective_compute(
                                        "AllGather", ALU.bypass,
                                        replica_groups=[list(range(cfg.NC))],
                                        ins=[bounceX[k].opt()],
                                        outs=[htblX[k].opt()])
                            elif w >= XW and ((w - XW) % 8 == 7
                                              or w == NWR - 1):
                                w0 = XW + ((w - XW) // 8) * 8
                                nc.sync.dma_start(
                                    bounceY[k][:].rearrange(
                                        "(t p) f -> p t f",
                                        p=128)[:, w0 - XW:w + 1 - XW, :],
                                    bsb[:, w0:w + 1, :])
                                if w == NWR - 1:
                                    nc.gpsimd.collective_compute(
                                        "AllGather", ALU.bypass,
                                        replica_groups=[list(range(cfg.NC))],
                                        ins=[bounceY[k].opt()],
                                        outs=[htblY[k].opt()])
                        if k == 2:
                            ps4 = psn.tile([128, 128], F32, tag="psn",
                                           name=f"ps4_{w}")
                            nc.tensor.matmul(ps4[0:64, :], w4h[:],
                                             h_cur[:, cols],
                                             start=True, stop=False)
                            nc.tensor.matmul(ps4[0:64, :], w4x[:],
                                             x_ownT[:, cols],
                                             start=False, stop=True)
                            z = nmp.tile([64, 128], F32, tag="z",
                                         name=f"z{w}")
                            nc.scalar.activation(z[:], ps4[0:64, :], AF.Relu,
                                                 bias=b4[:])
                            ps5 = psn.tile([128, 128], F32, tag="psn",
                                           name=f"ps5_{w}")
                            nc.tensor.matmul(ps5[0:1, :], w5[:], z[:],
                                             start=True, stop=True)
                            z5 = nmp.tile([1, 128], F32, tag="z5",
                                          name=f"z5_{w}")
                            nc.scalar.activation(z5[:], ps5[0:1, :],
                                                 AF.Identity,
                                                 bias=b5[0:1, :])
                            nc.sync.dma_start(out_ext.ap()[:, cols], z5[:])

                for j in range(NBLK + PRO):
                    if j < NBLK:
                        emit_a(j)
                        emit_b(j)
                    if j >= PRO:
                        emit_rest(j - PRO)

    nc.compile()
    return nc, gather_insts


def final_queue_map(nc, gather_insts):
    """Lane i (mod 8, final program order over Pool DMA insts) must keep a
    consistent SWDGE queue. Choose the lane->queue map to balance bytes."""
    lane_keys = [[] for _ in range(8)]
    lane_bytes = [0] * 8
    idx = 0
    for bb in nc.m.functions[0].blocks:
        for inst in bb.instructions:
            if type(inst).__name__ == "InstDMAGatherAnt":
                key = gather_insts.get(inst.name)
                assert key is not None, inst.name
                lane_keys[idx % 8].append(key)
                lane_bytes[idx % 8] += inst.num_idxs
                idx += 1
    order = sorted(range(8), key=lambda l: -lane_bytes[l])
    qb = [0.0] * 4
    qmap = {}
    for l in order:
        q = min(range(4), key=lambda i: qb[i])
        qb[q] += lane_bytes[l]
        for key in lane_keys[l]:
            qmap[key] = q
    return qmap


def build_kernel2(cfg, sched):
    nc1, gi1 = build_kernel(cfg, sched)
    qmap = final_queue_map(nc1, gi1)
    nc2, _ = build_kernel(cfg, sched, queue_map=qmap)
    return nc2


def assemble_output(cfg: Cfg, results):
    out = np.zeros(cfg.N, np.float32)
    for c, r in enumerate(results):
        lo, hi = c * cfg.NSH, min((c + 1) * cfg.NSH, cfg.N)
        out[lo:hi] = np.asarray(r["out"], np.float32).reshape(-1)[:hi - lo]
    return out


# ======================================================================
# Self-contained entry point: kernel(**inputs) -> np.ndarray [N] float32
# ======================================================================
from concourse.bass_utils import run_bass_kernel_spmd

_BUILD_CACHE = {}


def _get_nc(cfg, sched):
    key = tuple((tuple(a), tuple(b)) for a, b in sched)
    nc = _BUILD_CACHE.get(key)
    if nc is None:
        nc = build_kernel2(cfg, sched)
        _BUILD_CACHE[key] = nc
    return nc


def kernel(**inputs):
    cfg = FULL
    inp = {k: np.asarray(v) for k, v in inputs.items()}
    in_maps, sched = prep_inputs(cfg, inp)
    nc = _get_nc(cfg, sched)
    res = run_bass_kernel_spmd(nc, in_maps, core_ids=list(range(cfg.NC)),
                               trace=False)
    return assemble_output(cfg, res.results)



# revision 4
# speedup vs baseline: 1.2239x; 1.2239x over previous
"""GNN message-passing (ArtemisNet) distributed Bass kernel for 8 TRN2 cores, v3.

Strategy (v3 — hop-0 host-gathered stream + descriptor-exact gathers):
- dst-sharding: core c owns nodes [c*NSH, (c+1)*NSH). Edges assigned by dst.
- Hop 0: the gather of x[src] is fully resolved on the host into a
  slot-major, partition-major stream gx0[p, t, :] = [x|ea|1] of slot t*128+p,
  zero-padded. The kernel streams it with plain (HWDGE) dma_start — no
  GpSimd descriptor generation, no separate edge-attr matmuls.
- Hops 1-2: gather of h rows via dma_gather (int16 idx, X/Y table halves,
  chunk-major table layout). Per-(window,half) gather calls with runtime
  exact counts (num_idxs_reg via value_load): pad slots emit NO descriptors.
- Segment aggregation on TensorEngine: per 128-dst window, PSUM accumulates
  G_sub^T @ onehot_sub; pad slots have dstrel=-1 -> zero one-hot columns.
- Node-wise GEMMs feature-major; BN+ReLU folded into one ACT op.
- h tables republished per hop via per-half AllGathers, so the next hop's
  X gathers wait only on the X-half publish.
"""

import dataclasses
import numpy as np
import ml_dtypes

import concourse.bass as bass
import concourse.bacc as bacc
import concourse.tile as tile
import concourse.mybir as mybir

BF16 = mybir.dt.bfloat16
F32 = mybir.dt.float32
I16 = mybir.dt.int16
I32 = mybir.dt.int32
AF = mybir.ActivationFunctionType
ALU = mybir.AluOpType


@dataclasses.dataclass
class Cfg:
    N: int = 50000
    E: int = 800000
    NC: int = 8
    D_IN: int = 64
    D_E: int = 32
    H: int = 128
    EPS: float = 1e-5
    NSH: int = 6250          # nodes per core
    WSZ: int = 128           # dst window size
    NW: int = 50             # windows per core (NW*WSZ >= NSH)
    SHPAD: int = 6272        # padded shard rows in gather table (mult of 128)

    @property
    def NPAD(self):
        return self.NW * self.WSZ

    @property
    def NWR(self):
        return self.SHPAD // 128

    @property
    def XROWS(self):
        return (self.SHPAD // 128 * 31 // 49) * 128 if self.SHPAD > 256 else self.SHPAD // 2

    @property
    def YROWS(self):
        return self.SHPAD - self.XROWS

    @property
    def XW(self):
        return self.XROWS // 128

    def tblrow(self, s):
        """Vectorized: global node id -> (half(0=X,1=Y), table row)."""
        s = np.asarray(s, np.int64)
        sh = s // self.NSH
        r = s % self.NSH
        half = (r >= self.XROWS).astype(np.int64)
        row = np.where(half == 0, sh * self.XROWS + r,
                       sh * self.YROWS + (r - self.XROWS))
        return half, row


FULL = Cfg()


def _wrap_idx16(a):
    """[n] int -> [128, n//16] int16 (idx i at partition i%16, col i//16; tiled x8)."""
    n = a.shape[0]
    assert n % 16 == 0
    w = a.reshape(n // 16, 16).T.astype(np.int16)
    return np.tile(w, (8, 1)).copy()


def _slotmaj(a):
    """[TOT] -> [128, TOT//128] slot i at [i%128, i//128]."""
    t = a.shape[0]
    return np.ascontiguousarray(a.reshape(t // 128, 128).T)


def hop_counts(cfg: Cfg, src, dst, c, halves=True):
    sel = (dst >= c * cfg.NSH) & (dst < (c + 1) * cfg.NSH)
    s = src[sel].astype(np.int64)
    d = (dst[sel] - c * cfg.NSH).astype(np.int64)
    half = cfg.tblrow(s)[0] if halves else np.zeros_like(s)
    win = d // cfg.WSZ
    key = half * cfg.NW + win
    return np.bincount(key, minlength=2 * cfg.NW)


def compute_sched(cfg: Cfg, eidx):
    """Per-hop per-window sub-chunk counts (max over cores), SPMD-static."""
    sched = []
    for k in range(3):
        mx = np.zeros(2 * cfg.NW, np.int64)
        for c in range(cfg.NC):
            mx = np.maximum(mx, hop_counts(cfg, eidx[k][0], eidx[k][1], c,
                                           halves=(k > 0)))
        subsA = np.maximum(1, -(-mx[:cfg.NW] // 128))
        subsB = -(-mx[cfg.NW:] // 128)
        sched.append((subsA.astype(int), subsB.astype(int)))
    return sched


def sched_layout(cfg: Cfg, sub):
    """Slot bases per (half, window) from a hop schedule."""
    subsA, subsB = sub
    slotsA, slotsB = subsA * 128, subsB * 128
    a_tot = int(slotsA.sum())
    baseA = np.concatenate([[0], np.cumsum(slotsA)[:-1]])
    baseB = a_tot + np.concatenate([[0], np.cumsum(slotsB)[:-1]])
    tot = a_tot + int(slotsB.sum())
    return baseA, baseB, slotsA, slotsB, a_tot, tot


def prep_core_hop(cfg: Cfg, sub, src, dst, c, x=None, edge_attr=None):
    """Slot assignment for one (core, hop) under schedule `sub`.

    Pads are trailing within each (window, half) segment: gidx=-1, dstrel=-1.
    Returns per-(window,half) valid counts for runtime-exact gathers.

    Hop 0 (x is not None): all edges in the A "half"; instead of gather
    indices, emits the fully host-gathered slot stream gx[128, tot//128, 128].
    """
    hop0 = x is not None
    baseA, baseB, slotsA, slotsB, a_tot, tot = sched_layout(cfg, sub)
    sel = (dst >= c * cfg.NSH) & (dst < (c + 1) * cfg.NSH)
    s = src[sel].astype(np.int64)
    d = (dst[sel] - c * cfg.NSH).astype(np.int64)
    if hop0:
        half = np.zeros_like(s)
        tblrow = s  # unused
    else:
        half, tblrow = cfg.tblrow(s)
    win = d // cfg.WSZ

    dstrel = np.full(tot, -1.0, np.float32)

    order = np.lexsort((d, win, half))
    s_, d_, t_, h_, w_ = (v[order] for v in (s, d, tblrow, half, win))

    deg = np.bincount(d, minlength=cfg.NPAD).astype(np.float32)
    keys = h_ * cfg.NW + w_
    bnd = np.searchsorted(keys, np.arange(2 * cfg.NW + 1))
    cnts = (bnd[1:] - bnd[:-1]).astype(np.int64)
    slots_per = np.concatenate([slotsA, slotsB])
    assert (cnts <= slots_per).all(), f"slot overflow core {c}"
    seg_base = np.concatenate([baseA, baseB])
    pos = (seg_base[keys] + np.arange(len(keys)) - bnd[keys]).astype(np.int64)
    dstrel[pos] = (d_ - w_ * cfg.WSZ).astype(np.float32)

    invdeg = (1.0 / np.maximum(deg, 1.0)).astype(np.float32)
    out = {
        "inv": np.broadcast_to(invdeg.astype(ml_dtypes.bfloat16),
                               (128, cfg.NPAD)).copy(),
        "dstrel": _slotmaj(dstrel.astype(ml_dtypes.bfloat16)),
    }

    if hop0:
        gx = np.zeros((tot, 128), np.float32)
        gx[pos, :cfg.D_IN] = x[s_]
        gx[pos, cfg.D_IN:cfg.D_IN + cfg.D_E] = edge_attr[sel][order]
        gx[pos, cfg.D_IN + cfg.D_E] = 1.0
        out["gx"] = np.ascontiguousarray(
            gx.reshape(tot // 128, 128, 128).transpose(1, 0, 2)
        ).astype(ml_dtypes.bfloat16)
        return out

    gidx = np.full(tot, -1, np.int64)
    gidx[pos] = t_

    # zero-count segments with scheduled slots: keep one valid dummy desc
    # (row 0, dstrel=-1) so num_idxs_reg >= 1 everywhere.
    for seg in range(2 * cfg.NW):
        if cnts[seg] == 0 and slots_per[seg] > 0:
            gidx[seg_base[seg]] = 0
            cnts[seg] = 1

    cnt128 = np.zeros(128, np.int32)
    cnt128[:cfg.NW] = cnts[:cfg.NW]          # A counts
    cnt128[64:64 + cfg.NW] = cnts[cfg.NW:]   # B counts
    out["gidx"] = _wrap_idx16(gidx)
    out["cnt"] = cnt128.reshape(1, 128)
    return out


def prep_inputs(cfg: Cfg, inp):
    """Full-host preprocessing: returns in_maps (list of dicts, one per core)."""
    x = np.asarray(inp["x"], np.float32)
    H, D_IN, D_E = cfg.H, cfg.D_IN, cfg.D_E

    W1 = np.asarray(inp["W1"], np.float32)  # [H, D_IN+D_E]
    w1c = np.zeros((D_IN + D_E + 1, H), np.float32)
    w1c[:D_IN] = W1[:, :D_IN].T
    w1c[D_IN:D_IN + D_E] = W1[:, D_IN:].T
    w1c[D_IN + D_E] = np.asarray(inp["b1"], np.float32)
    w1c = w1c.astype(ml_dtypes.bfloat16)

    def bn_fold(g, be, m, v, blin=None):
        g, be, m, v = (np.asarray(inp[k], np.float32) for k in (g, be, m, v))
        gam = g / np.sqrt(v + cfg.EPS)
        bet = be - m * gam
        if blin is not None:
            bet = bet + gam * np.asarray(inp[blin], np.float32)
        return gam.reshape(-1, 1), bet.reshape(-1, 1)

    sc1, bs1 = bn_fold("g1", "be1", "m1", "v1")
    sc2, bs2 = bn_fold("g2", "be2", "m2", "v2", "bl2")
    sc3, bs3 = bn_fold("g3", "be3", "m3", "v3", "bl3")

    W4 = np.asarray(inp["W4"], np.float32)  # [64, H+D_IN]
    w4h = W4[:, :H].T.astype(ml_dtypes.bfloat16)         # [H, 64]
    w4x = W4[:, H:].T.astype(ml_dtypes.bfloat16)         # [D_IN, 64]
    b4 = np.asarray(inp["b4"], np.float32).reshape(-1, 1)
    w5 = np.asarray(inp["W5"], np.float32).T             # [64, 1]
    b5 = np.asarray(inp["b5"], np.float32).reshape(1, 1)

    iota = np.broadcast_to(np.arange(128, dtype=np.float32), (128, 128)
                           ).astype(ml_dtypes.bfloat16)
    ident_bf = np.eye(128, dtype=ml_dtypes.bfloat16)

    shared = {
        "w1c": w1c,
        "w2l": np.asarray(inp["Wl2"], np.float32).T.astype(ml_dtypes.bfloat16),
        "w2r": np.asarray(inp["Wr2"], np.float32).T.astype(ml_dtypes.bfloat16),
        "w3l": np.asarray(inp["Wl3"], np.float32).T.astype(ml_dtypes.bfloat16),
        "w3r": np.asarray(inp["Wr3"], np.float32).T.astype(ml_dtypes.bfloat16),
        "sc1": sc1, "bs1": bs1, "sc2": sc2, "bs2": bs2, "sc3": sc3, "bs3": bs3,
        "w4h": w4h, "w4x": w4x, "b4": b4, "w5": w5, "b5": b5,
        "iota": iota, "ident_bf": ident_bf,
    }

    eidx = [np.asarray(inp[f"edge_index_{k}"]) for k in range(3)]
    ea0 = np.asarray(inp["edge_attr_0"], np.float32)
    sched = compute_sched(cfg, eidx)

    in_maps = []
    for c in range(cfg.NC):
        m = dict(shared)
        # x_ownT bf16 [D_IN, NPAD]
        xo = np.zeros((cfg.NPAD, D_IN), np.float32)
        lo, hi = c * cfg.NSH, min((c + 1) * cfg.NSH, cfg.N)
        xo[:hi - lo] = x[lo:hi]
        m["x_ownT"] = xo.T.astype(ml_dtypes.bfloat16).copy()
        for k in range(3):
            p = prep_core_hop(cfg, sched[k], eidx[k][0], eidx[k][1], c,
                              x=x if k == 0 else None,
                              edge_attr=ea0 if k == 0 else None)
            m[f"dstrel{k}"] = p["dstrel"]
            m[f"inv{k}"] = p["inv"]
            if k == 0:
                m["gx0"] = p["gx"]
            else:
                m[f"gidx{k}"] = p["gidx"]
                m[f"cnt{k}"] = p["cnt"]
        in_maps.append(m)
    return in_maps, sched


def build_kernel(cfg: Cfg, sched, queue_map=None):
    gather_insts = {}  # inst name -> key

    nc = bacc.Bacc("TRN2", target_bir_lowering=False, debug=False,
                   num_devices=cfg.NC, num_swdge_queues=4,
                   dynamic_dma_scratch_size=16384)
    H, D_IN, D_E = cfg.H, cfg.D_IN, cfg.D_E
    DXE = D_IN + D_E
    NW = cfg.NW
    WPB = 2 if NW % 2 == 0 else 1
    NBLK = NW // WPB
    lay = [sched_layout(cfg, sub) for sub in sched]
    MAXSUBA0 = int(sched[0][0].max())
    MAXSUBA = max(int(sub[0].max()) for sub in sched[1:])
    MAXSUBB = max(max(int(sub[1].max()), 1) for sub in sched[1:])
    MA = max(WPB * MAXSUBA, MAXSUBA0)
    MB = WPB * MAXSUBB

    P = {}

    def par(name, shape, dt=F32, out=False):
        P[name] = nc.declare_dram_parameter(name, list(shape), dt, isOutput=out)
        return P[name]

    par("gx0", (128, lay[0][5] // 128, 128), BF16)
    par("x_ownT", (D_IN, cfg.NPAD), BF16)
    for k in range(3):
        par(f"dstrel{k}", (128, lay[k][5] // 128), BF16)
        par(f"inv{k}", (128, cfg.NPAD), BF16)
        if k > 0:
            par(f"gidx{k}", (128, lay[k][5] // 16), I16)
            par(f"cnt{k}", (1, 128), I32)
    par("w1c", (DXE + 1, H), BF16)
    par("w2l", (H, H), BF16); par("w2r", (H, H), BF16)
    par("w3l", (H, H), BF16); par("w3r", (H, H), BF16)
    for nm in ("sc1", "bs1", "sc2", "bs2", "sc3", "bs3"):
        par(nm, (H, 1))
    par("w4h", (H, 64), BF16); par("w4x", (D_IN, 64), BF16)
    par("b4", (64, 1)); par("w5", (64, 1)); par("b5", (1, 1))
    par("iota", (128, 128), BF16)
    par("ident_bf", (128, 128), BF16)
    out_ext = par("out", (1, cfg.NPAD), out=True)

    with tile.TileContext(nc) as tc:
        with (
            tc.tile_pool(name="const", bufs=1) as cp,
            tc.tile_pool(name="invp", bufs=2) as invp,
            tc.tile_pool(name="cnp", bufs=3) as cnp,
            tc.tile_pool(name="ohp", bufs=4) as ohp,
            tc.tile_pool(name="ip", bufs=4) as ip,
            tc.tile_pool(name="hp", bufs=2) as hp,
            tc.tile_pool(name="nmp", bufs=6) as nmp,
            tc.tile_pool(name="pse", bufs=3, space="PSUM") as pse,
            tc.tile_pool(name="psn", bufs=5, space="PSUM") as psn,
            tc.tile_pool(name="dram", bufs=1, space="DRAM") as dp,
        ):
            def ld(name, dt=F32):
                t = cp.tile(list(P[name].shape), dt, tag=name)
                nc.scalar.dma_start(t[:], P[name].ap())
                return t

            w1c = ld("w1c", BF16)
            w2l = ld("w2l", BF16); w3l = ld("w3l", BF16)
            w2r = ld("w2r", BF16); w3r = ld("w3r", BF16)
            sc = [ld(f"sc{k}") for k in (1, 2, 3)]
            bs = [ld(f"bs{k}") for k in (1, 2, 3)]
            w4h = ld("w4h", BF16); w4x = ld("w4x", BF16)
            b4 = ld("b4"); w5 = ld("w5"); b5 = ld("b5")
            iota = ld("iota", BF16)
            ident_bf = ld("ident_bf", BF16)
            x_ownT = ld("x_ownT", BF16)
            iota_big = cp.tile([128, MA, 128], BF16, tag="iota_big")
            nc.vector.tensor_copy(
                iota_big[:],
                iota[:].rearrange("p (o f) -> p o f", o=1).broadcast_to(
                    [128, MA, 128]))

            # persistent gather ring buffers: pad slots skip their DMA and
            # must read as finite values for the masked matmul, so the rings
            # are zeroed once here and then only ever overwritten by gathers
            # (or hop-0's streamed blocks, which cover every slot they read).
            RING_A = 6
            RING_B = 6
            gbufA = cp.tile([128, RING_A * MA, 128], BF16, tag="gbufA")
            gbufB = cp.tile([128, RING_B * MB, 128], BF16, tag="gbufB")
            for rp in range(RING_A):
                nc.vector.memset(gbufA[:, rp * MA:(rp + 1) * MA, :], 0.0)
            for rp in range(RING_B):
                nc.vector.memset(gbufB[:, rp * MB:(rp + 1) * MB, :], 0.0)

            htblX = [dp.tile([cfg.NC * cfg.XROWS, 128], BF16,
                             name=f"htblX{k}", tag=f"htblX{k}",
                             addr_space="Shared") for k in range(2)]
            htblY = [dp.tile([cfg.NC * cfg.YROWS, 128], BF16,
                             name=f"htblY{k}", tag=f"htblY{k}",
                             addr_space="Shared") for k in range(2)]
            bounceX = [dp.tile([cfg.XROWS, 128], BF16, name=f"bounceX{k}",
                               tag=f"bounceX{k}") for k in range(2)]
            bounceY = [dp.tile([cfg.YROWS, 128], BF16, name=f"bounceY{k}",
                               tag=f"bounceY{k}") for k in range(2)]

            # tiny warm-up collective: absorbs first-call AG overhead
            wub = dp.tile([128, 128], BF16, tag="wub")
            wuo = dp.tile([cfg.NC * 128, 128], BF16, tag="wuo",
                          addr_space="Shared")
            wz = cp.tile([128, 128], BF16, tag="wz")
            nc.vector.memset(wz[:], 0.0)
            nc.sync.dma_start(wub[:], wz[:])
            nc.gpsimd.collective_compute(
                "AllGather", ALU.bypass,
                replica_groups=[list(range(cfg.NC))],
                ins=[wub.opt()], outs=[wuo.opt()])

            h_prev = None
            h_cur = None
            cnt_regs = [nc.gpsimd.alloc_register(f"cntreg{i}")
                        for i in range(12)]
            creg_ctr = [0]
            ring_ctr = {"A": 0, "B": 0}

            def load_cnt(ap):
                r = cnt_regs[creg_ctr[0] % 12]
                creg_ctr[0] += 1
                nc.gpsimd.reg_load(r, ap)
                return r

            def reg_gather(inst, key):
                gather_insts[inst.ins.name] = key

            for k in range(3):
                baseA, baseB, slotsA, slotsB, a_tot, tot = lay[k]
                subsA, subsB = sched[k]
                WPBk = 1 if k == 0 else WPB
                NBLKk = NW // WPBk
                PRO = 4 if k else 3   # gather/stream lookahead depth (blocks)
                if k > 0:
                    tblX = htblX[k - 1][:]
                    tblY = htblY[k - 1][:]
                inv = invp.tile([128, cfg.NPAD], BF16, tag="inv_rep",
                                name=f"invt{k}")
                nc.scalar.dma_start(inv[:], P[f"inv{k}"].ap())
                if k > 0:
                    cntT = cnp.tile([1, 128], I32, tag="cntT", name=f"cntT{k}")
                    nc.sync.dma_start(cntT[:], P[f"cnt{k}"].ap())

                h_prev = h_cur
                h_cur = hp.tile([128, cfg.NPAD], BF16, tag="h", name=f"h{k}")
                if k < 2:
                    bsb = cp.tile([128, cfg.NWR, 128], BF16,
                                  tag="bsb", name=f"bsb{k}")
                fdim = DXE + 1 if k == 0 else 128
                state = {}

                def emit_a(j, k=k, state=state,
                           subsA=subsA, subsB=subsB, baseA=baseA, baseB=baseB,
                           WPBk=WPBk):
                    ws = list(range(j * WPBk, (j + 1) * WPBk))
                    nsa = [int(subsA[w]) for w in ws]
                    nsb = [int(subsB[w]) for w in ws]
                    nA = 128 * sum(nsa)
                    nB = 128 * sum(nsb)
                    sA0 = int(baseA[ws[0]])
                    sB0 = int(baseB[ws[0]])
                    rp = ring_ctr["A"] % RING_A
                    ring_ctr["A"] += 1
                    ga = gbufA[:, rp * MA:rp * MA + sum(nsa), :]
                    if k == 0:
                        nc.sync.dma_start(
                            ga[:], P["gx0"].ap()[:, sA0 // 128:
                                                 (sA0 + nA) // 128, :])
                        state[j] = (ga, None, nsa, nsb, nA, nB, sA0, sB0)
                        return
                    gia = ip.tile([128, nA // 16], I16, tag="gia",
                                  name=f"gia{k}_{j}")
                    nc.sync.dma_start(
                        gia[:], P[f"gidx{k}"].ap()[:, sA0 // 16:
                                                   (sA0 + nA) // 16])
                    for wl in range(WPBk):
                        w = ws[wl]
                        offa = sum(nsa[:wl])
                        na_w = nsa[wl] * 128
                        cva = load_cnt(cntT[0:1, w:w + 1])
                        keyA = (k, j, wl, 0)
                        reg_gather(nc.gpsimd.dma_gather(
                            ga[:, offa:offa + nsa[wl], :], tblX,
                            gia[:, (offa * 128) // 16:
                                (offa * 128 + na_w) // 16],
                            na_w, cva, 128, single_packet=False,
                            queue_num=(queue_map or {}).get(keyA, 0)), keyA)
                    state[j] = (ga, None, nsa, nsb, nA, nB, sA0, sB0)

                def emit_b(j, k=k, state=state,
                           subsB=subsB, baseB=baseB, WPBk=WPBk):
                    if k == 0:
                        return
                    ga, _, nsa, nsb, nA, nB, sA0, sB0 = state[j]
                    ws = list(range(j * WPBk, (j + 1) * WPBk))
                    if nB == 0:
                        return
                    rp = ring_ctr["B"] % RING_B
                    ring_ctr["B"] += 1
                    gb = gbufB[:, rp * MB:rp * MB + max(sum(nsb), 1), :]
                    gib = ip.tile([128, nB // 16], I16, tag="gib",
                                  name=f"gib{k}_{j}")
                    nc.sync.dma_start(
                        gib[:], P[f"gidx{k}"].ap()[:, sB0 // 16:
                                                   (sB0 + nB) // 16])
                    for wl in range(WPBk):
                        w = ws[wl]
                        if nsb[wl] > 0:
                            offb = sum(nsb[:wl])
                            nb_w = nsb[wl] * 128
                            cvb = load_cnt(cntT[0:1, 64 + w:64 + w + 1])
                            keyB = (k, j, wl, 1)
                            reg_gather(nc.gpsimd.dma_gather(
                                gb[:, offb:offb + nsb[wl], :], tblY,
                                gib[:, (offb * 128) // 16:
                                    (offb * 128 + nb_w) // 16],
                                nb_w, cvb, 128, single_packet=False,
                                queue_num=(queue_map or {}).get(keyB, 0)), keyB)
                    state[j] = (ga, gb, nsa, nsb, nA, nB, sA0, sB0)

                def emit_rest(j, k=k, state=state, inv=inv, h_cur=h_cur,
                              h_prev=h_prev, fdim=fdim, WPBk=WPBk,
                              bsb=(bsb if k < 2 else None)):
                    ga, gb, nsa, nsb, nA, nB, sA0, sB0 = state.pop(j)
                    ws = list(range(j * WPBk, (j + 1) * WPBk))
                    oa = ohp.tile([128, sum(nsa), 128], BF16, tag="oa",
                                  name=f"oa{k}_{j}")
                    dra = ip.tile([128, sum(nsa)], BF16, tag="dra",
                                  name=f"dra{k}_{j}")
                    nc.sync.dma_start(
                        dra[:], P[f"dstrel{k}"].ap()[:, sA0 // 128:
                                                     (sA0 + nA) // 128])
                    nc.vector.tensor_tensor(
                        oa[:], iota_big[:, 0:sum(nsa), :],
                        dra[:].rearrange("p (s o) -> p s o", o=1).broadcast_to(
                            [128, sum(nsa), 128]),
                        ALU.is_equal)
                    ob = None
                    if nB > 0:
                        ob = ohp.tile([128, sum(nsb), 128], BF16, tag="ob",
                                      name=f"ob{k}_{j}")
                        drb = ip.tile([128, sum(nsb)], BF16, tag="drb",
                                      name=f"drb{k}_{j}")
                        nc.sync.dma_start(
                            drb[:], P[f"dstrel{k}"].ap()[:, sB0 // 128:
                                                         (sB0 + nB) // 128])
                        nc.vector.tensor_tensor(
                            ob[:], iota_big[:, 0:sum(nsb), :],
                            drb[:].rearrange(
                                "p (s o) -> p s o", o=1).broadcast_to(
                                [128, sum(nsb), 128]),
                            ALU.is_equal)

                    for wl in range(WPBk):
                        w = ws[wl]
                        offa = sum(nsa[:wl])
                        offb = sum(nsb[:wl])
                        cols = slice(w * 128, (w + 1) * 128)
                        nmm = nsa[wl] + nsb[wl]
                        mi = 0
                        ps = pse.tile([128, 128], F32, tag="ps",
                                      name=f"ps{k}_{w}")
                        for t in range(nsa[wl]):
                            nc.tensor.matmul(
                                ps[0:fdim, :], ga[:, offa + t, 0:fdim],
                                oa[:, offa + t, :],
                                start=(mi == 0), stop=(mi == nmm - 1))
                            mi += 1
                        for t in range(nsb[wl]):
                            nc.tensor.matmul(
                                ps[0:fdim, :], gb[:, offb + t, 0:fdim],
                                ob[:, offb + t, :],
                                start=(mi == 0), stop=(mi == nmm - 1))
                            mi += 1
                        rhs = nmp.tile([128, 128], BF16, tag="rhs",
                                       name=f"rhs{k}_{w}")
                        nc.vector.tensor_tensor(rhs[0:fdim, :], ps[0:fdim, :],
                                                inv[0:fdim, cols], ALU.mult)
                        ps2 = psn.tile([128, 128], F32, tag="psn",
                                       name=f"ps2{k}_{w}")
                        if k == 0:
                            nc.tensor.matmul(ps2[:], w1c[:], rhs[0:fdim, :],
                                             start=True, stop=True)
                            tmp = nmp.tile([128, 128], F32, tag="tmp",
                                           name=f"tmp{w}")
                            nc.scalar.activation(tmp[:], ps2[:], AF.Relu)
                            nc.scalar.activation(h_cur[:, cols], tmp[:],
                                                 AF.Relu, bias=bs[0][:],
                                                 scale=sc[0][:])
                        else:
                            wl_ = w2l if k == 1 else w3l
                            wr_ = w2r if k == 1 else w3r
                            nc.tensor.matmul(ps2[:], wl_[:], rhs[:],
                                             start=True, stop=False)
                            nc.tensor.matmul(ps2[:], wr_[:], h_prev[:, cols],
                                             start=False, stop=True)
                            nc.scalar.activation(h_cur[:, cols], ps2[:],
                                                 AF.Relu, bias=bs[k][:],
                                                 scale=sc[k][:])
                        if k < 2 and w < cfg.NWR:
                            pstr = psn.tile([128, 128], BF16, tag="psn",
                                            name=f"pstr{k}_{w}")
                            nc.tensor.transpose(pstr[:], h_cur[:, cols],
                                                ident_bf[:])
                            nc.scalar.activation(bsb[:, w, :], pstr[:],
                                                 AF.Copy)
                            XW = cfg.XW
                            NWR = cfg.NWR
                            if w < XW and (w % 8 == 7 or w == XW - 1):
                                w0 = (w // 8) * 8
                                nc.sync.dma_start(
                                    bounceX[k][:].rearrange(
                                        "(t p) f -> p t f",
                                        p=128)[:, w0:w + 1, :],
                                    bsb[:, w0:w + 1, :])
                                if w == XW - 1:
                                    nc.gpsimd.collective_compute(
                                        "AllGather", ALU.bypass,
                                        replica_groups=[list(range(cfg.NC))],
                                        ins=[bounceX[k].opt()],
                                        outs=[htblX[k].opt()])
                            elif w >= XW and ((w - XW) % 8 == 7
                                              or w == NWR - 1):
                                w0 = XW + ((w - XW) // 8) * 8
                                nc.sync.dma_start(
                                    bounceY[k][:].rearrange(
                                        "(t p) f -> p t f",
                                        p=128)[:, w0 - XW:w + 1 - XW, :],
                                    bsb[:, w0:w + 1, :])
                                if w == NWR - 1:
                                    nc.gpsimd.collective_compute(
                                        "AllGather", ALU.bypass,
                                        replica_groups=[list(range(cfg.NC))],
                                        ins=[bounceY[k].opt()],
                                        outs=[htblY[k].opt()])
                        if k == 2:
                            ps4 = psn.tile([128, 128], F32, tag="psn",
                                           name=f"ps4_{w}")
                            nc.tensor.matmul(ps4[0:64, :], w4h[:],
                                             h_cur[:, cols],
                                             start=True, stop=False)
                            nc.tensor.matmul(ps4[0:64, :], w4x[:],
                                             x_ownT[:, cols],
                                             start=False, stop=True)
                            z = nmp.tile([64, 128], F32, tag="z",
                                         name=f"z{w}")
                            nc.scalar.activation(z[:], ps4[0:64, :], AF.Relu,
                                                 bias=b4[:])
                            ps5 = psn.tile([128, 128], F32, tag="psn",
                                           name=f"ps5_{w}")
                            nc.tensor.matmul(ps5[0:1, :], w5[:], z[:],
                                             start=True, stop=True)
                            z5 = nmp.tile([1, 128], F32, tag="z5",
                                          name=f"z5_{w}")
                            nc.scalar.activation(z5[:], ps5[0:1, :],
                                                 AF.Identity,
                                                 bias=b5[0:1, :])
                            nc.sync.dma_start(out_ext.ap()[:, cols], z5[:])

                for j in range(NBLKk + PRO):
                    if j < NBLKk:
                        emit_a(j)
                        emit_b(j)
                    if j >= PRO:
                        emit_rest(j - PRO)

    nc.compile()
    return nc, gather_insts


def final_queue_map(nc, gather_insts):
    """Lane i (mod 8, final program order over Pool DMA insts) must keep a
    consistent SWDGE queue. Choose the lane->queue map to balance bytes."""
    lane_keys = [[] for _ in range(8)]
    lane_bytes = [0] * 8
    idx = 0
    for bb in nc.m.functions[0].blocks:
        for inst in bb.instructions:
            if type(inst).__name__ == "InstDMAGatherAnt":
                key = gather_insts.get(inst.name)
                assert key is not None, inst.name
                lane_keys[idx % 8].append(key)
                lane_bytes[idx % 8] += inst.num_idxs
                idx += 1
    order = sorted(range(8), key=lambda l: -lane_bytes[l])
    qb = [0.0] * 4
    qmap = {}
    for l in order:
        q = min(range(4), key=lambda i: qb[i])
        qb[q] += lane_bytes[l]
        for key in lane_keys[l]:
            qmap[key] = q
    return qmap


def build_kernel2(cfg, sched):
    nc1, gi1 = build_kernel(cfg, sched)
    qmap = final_queue_map(nc1, gi1)
    nc2, _ = build_kernel(cfg, sched, queue_map=qmap)
    return nc2


def assemble_output(cfg: Cfg, results):
    out = np.zeros(cfg.N, np.float32)
    for c, r in enumerate(results):
        lo, hi = c * cfg.NSH, min((c + 1) * cfg.NSH, cfg.N)
        out[lo:hi] = np.asarray(r["out"], np.float32).reshape(-1)[:hi - lo]
    return out


# ======================================================================
# Self-contained entry point: kernel(**inputs) -> np.ndarray [N] float32
# ======================================================================
from concourse.bass_utils import run_bass_kernel_spmd

_BUILD_CACHE = {}


def _get_nc(cfg, sched):
    key = tuple((tuple(a), tuple(b)) for a, b in sched)
    nc = _BUILD_CACHE.get(key)
    if nc is None:
        nc = build_kernel2(cfg, sched)
        _BUILD_CACHE[key] = nc
    return nc


def kernel(**inputs):
    cfg = FULL
    inp = {k: np.asarray(v) for k, v in inputs.items()}
    in_maps, sched = prep_inputs(cfg, inp)
    nc = _get_nc(cfg, sched)
    res = run_bass_kernel_spmd(nc, in_maps, core_ids=list(range(cfg.NC)),
                               trace=False)
    return assemble_output(cfg, res.results)


# revision 5
# speedup vs baseline: 1.3610x; 1.1121x over previous
"""GNN message-passing (ArtemisNet) distributed Bass kernel for 8 TRN2 cores, v3.

Strategy (v3 — hop-0 host-gathered stream + descriptor-exact gathers):
- dst-sharding: core c owns nodes [c*NSH, (c+1)*NSH). Edges assigned by dst.
- Hop 0: the gather of x[src] is fully resolved on the host into a
  slot-major, partition-major stream gx0[p, t, :] = [x|ea|1] of slot t*128+p,
  zero-padded. The kernel streams it with plain (HWDGE) dma_start — no
  GpSimd descriptor generation, no separate edge-attr matmuls.
- Hops 1-2: gather of h rows via dma_gather (int16 idx, X/Y table halves,
  chunk-major table layout). Per-(window,half) gather calls with runtime
  exact counts (num_idxs_reg via value_load): pad slots emit NO descriptors.
- Segment aggregation on TensorEngine: per 128-dst window, PSUM accumulates
  G_sub^T @ onehot_sub; pad slots have dstrel=-1 -> zero one-hot columns.
- Node-wise GEMMs feature-major; BN+ReLU folded into one ACT op.
- h tables republished per hop via per-half AllGathers, so the next hop's
  X gathers wait only on the X-half publish.
"""

import dataclasses
import numpy as np
import ml_dtypes

import concourse.bass as bass
import concourse.bacc as bacc
import concourse.tile as tile
import concourse.mybir as mybir

BF16 = mybir.dt.bfloat16
F32 = mybir.dt.float32
I16 = mybir.dt.int16
I32 = mybir.dt.int32
AF = mybir.ActivationFunctionType
ALU = mybir.AluOpType


@dataclasses.dataclass
class Cfg:
    N: int = 50000
    E: int = 800000
    NC: int = 8
    D_IN: int = 64
    D_E: int = 32
    H: int = 128
    EPS: float = 1e-5
    NSH: int = 6250          # nodes per core
    WSZ: int = 128           # dst window size
    NW: int = 50             # windows per core (NW*WSZ >= NSH)
    SHPAD: int = 6272        # padded shard rows in gather table (mult of 128)

    @property
    def NPAD(self):
        return self.NW * self.WSZ

    @property
    def NWR(self):
        return self.SHPAD // 128

    @property
    def XROWS(self):
        return (self.SHPAD // 128 * 31 // 49) * 128 if self.SHPAD > 256 else self.SHPAD // 2

    @property
    def YROWS(self):
        return self.SHPAD - self.XROWS

    @property
    def XW(self):
        return self.XROWS // 128

    def tblrow(self, s):
        """Vectorized: global node id -> (half(0=X,1=Y), table row)."""
        s = np.asarray(s, np.int64)
        sh = s // self.NSH
        r = s % self.NSH
        half = (r >= self.XROWS).astype(np.int64)
        row = np.where(half == 0, sh * self.XROWS + r,
                       sh * self.YROWS + (r - self.XROWS))
        return half, row


FULL = Cfg()


def _wrap_idx16(a):
    """[n] int -> [128, n//16] int16 (idx i at partition i%16, col i//16; tiled x8)."""
    n = a.shape[0]
    assert n % 16 == 0
    w = a.reshape(n // 16, 16).T.astype(np.int16)
    return np.tile(w, (8, 1)).copy()


def _slotmaj(a):
    """[TOT] -> [128, TOT//128] slot i at [i%128, i//128]."""
    t = a.shape[0]
    return np.ascontiguousarray(a.reshape(t // 128, 128).T)


def hop_counts(cfg: Cfg, src, dst, c, halves=True):
    sel = (dst >= c * cfg.NSH) & (dst < (c + 1) * cfg.NSH)
    s = src[sel].astype(np.int64)
    d = (dst[sel] - c * cfg.NSH).astype(np.int64)
    half = cfg.tblrow(s)[0] if halves else np.zeros_like(s)
    win = d // cfg.WSZ
    key = half * cfg.NW + win
    return np.bincount(key, minlength=2 * cfg.NW)


def compute_sched(cfg: Cfg, eidx):
    """Per-hop per-window sub-chunk counts (max over cores), SPMD-static."""
    sched = []
    for k in range(3):
        mx = np.zeros(2 * cfg.NW, np.int64)
        for c in range(cfg.NC):
            mx = np.maximum(mx, hop_counts(cfg, eidx[k][0], eidx[k][1], c,
                                           halves=(k > 0)))
        subsA = np.maximum(1, -(-mx[:cfg.NW] // 128))
        subsB = -(-mx[cfg.NW:] // 128)
        sched.append((subsA.astype(int), subsB.astype(int)))
    return sched


def sched_layout(cfg: Cfg, sub):
    """Slot bases per (half, window) from a hop schedule."""
    subsA, subsB = sub
    slotsA, slotsB = subsA * 128, subsB * 128
    a_tot = int(slotsA.sum())
    baseA = np.concatenate([[0], np.cumsum(slotsA)[:-1]])
    baseB = a_tot + np.concatenate([[0], np.cumsum(slotsB)[:-1]])
    tot = a_tot + int(slotsB.sum())
    return baseA, baseB, slotsA, slotsB, a_tot, tot


def prep_core_hop(cfg: Cfg, sub, src, dst, c, x=None, edge_attr=None):
    """Slot assignment for one (core, hop) under schedule `sub`.

    Pads are trailing within each (window, half) segment: gidx=-1, dstrel=-1.
    Returns per-(window,half) valid counts for runtime-exact gathers.

    Hop 0 (x is not None): all edges in the A "half"; instead of gather
    indices, emits the fully host-gathered slot stream gx[128, tot//128, 128].
    """
    hop0 = x is not None
    baseA, baseB, slotsA, slotsB, a_tot, tot = sched_layout(cfg, sub)
    sel = (dst >= c * cfg.NSH) & (dst < (c + 1) * cfg.NSH)
    s = src[sel].astype(np.int64)
    d = (dst[sel] - c * cfg.NSH).astype(np.int64)
    if hop0:
        half = np.zeros_like(s)
        tblrow = s  # unused
    else:
        half, tblrow = cfg.tblrow(s)
    win = d // cfg.WSZ

    dstrel = np.full(tot, -1.0, np.float32)

    order = np.lexsort((d, win, half))
    s_, d_, t_, h_, w_ = (v[order] for v in (s, d, tblrow, half, win))

    deg = np.bincount(d, minlength=cfg.NPAD).astype(np.float32)
    keys = h_ * cfg.NW + w_
    bnd = np.searchsorted(keys, np.arange(2 * cfg.NW + 1))
    cnts = (bnd[1:] - bnd[:-1]).astype(np.int64)
    slots_per = np.concatenate([slotsA, slotsB])
    assert (cnts <= slots_per).all(), f"slot overflow core {c}"
    seg_base = np.concatenate([baseA, baseB])
    pos = (seg_base[keys] + np.arange(len(keys)) - bnd[keys]).astype(np.int64)
    dstrel[pos] = (d_ - w_ * cfg.WSZ).astype(np.float32)

    invdeg = (1.0 / np.maximum(deg, 1.0)).astype(np.float32)
    out = {
        "inv": np.broadcast_to(invdeg.astype(ml_dtypes.bfloat16),
                               (128, cfg.NPAD)).copy(),
        "dstrel": _slotmaj(dstrel.astype(ml_dtypes.bfloat16)),
    }

    if hop0:
        gx = np.zeros((tot, 128), np.float32)
        gx[pos, :cfg.D_IN] = x[s_]
        gx[pos, cfg.D_IN:cfg.D_IN + cfg.D_E] = edge_attr[sel][order]
        gx[pos, cfg.D_IN + cfg.D_E] = 1.0
        out["gx"] = np.ascontiguousarray(
            gx.reshape(tot // 128, 128, 128).transpose(1, 0, 2)
        ).astype(ml_dtypes.bfloat16)
        return out

    gidx = np.full(tot, -1, np.int64)
    gidx[pos] = t_

    # zero-count segments with scheduled slots: keep one valid dummy desc
    # (row 0, dstrel=-1) so num_idxs_reg >= 1 everywhere.
    for seg in range(2 * cfg.NW):
        if cnts[seg] == 0 and slots_per[seg] > 0:
            gidx[seg_base[seg]] = 0
            cnts[seg] = 1

    cnt128 = np.zeros(128, np.int32)
    cnt128[:cfg.NW] = cnts[:cfg.NW]          # A counts
    cnt128[64:64 + cfg.NW] = cnts[cfg.NW:]   # B counts
    out["gidx"] = _wrap_idx16(gidx)
    out["cnt"] = cnt128.reshape(1, 128)
    return out


def prep_inputs(cfg: Cfg, inp):
    """Full-host preprocessing: returns in_maps (list of dicts, one per core)."""
    x = np.asarray(inp["x"], np.float32)
    H, D_IN, D_E = cfg.H, cfg.D_IN, cfg.D_E

    W1 = np.asarray(inp["W1"], np.float32)  # [H, D_IN+D_E]
    w1c = np.zeros((D_IN + D_E + 1, H), np.float32)
    w1c[:D_IN] = W1[:, :D_IN].T
    w1c[D_IN:D_IN + D_E] = W1[:, D_IN:].T
    w1c[D_IN + D_E] = np.asarray(inp["b1"], np.float32)
    w1c = w1c.astype(ml_dtypes.bfloat16)

    def bn_fold(g, be, m, v, blin=None):
        g, be, m, v = (np.asarray(inp[k], np.float32) for k in (g, be, m, v))
        gam = g / np.sqrt(v + cfg.EPS)
        bet = be - m * gam
        if blin is not None:
            bet = bet + gam * np.asarray(inp[blin], np.float32)
        return gam.reshape(-1, 1), bet.reshape(-1, 1)

    sc1, bs1 = bn_fold("g1", "be1", "m1", "v1")
    sc2, bs2 = bn_fold("g2", "be2", "m2", "v2", "bl2")
    sc3, bs3 = bn_fold("g3", "be3", "m3", "v3", "bl3")

    W4 = np.asarray(inp["W4"], np.float32)  # [64, H+D_IN]
    w4h = W4[:, :H].T.astype(ml_dtypes.bfloat16)         # [H, 64]
    w4x = W4[:, H:].T.astype(ml_dtypes.bfloat16)         # [D_IN, 64]
    b4 = np.asarray(inp["b4"], np.float32).reshape(-1, 1)
    w5 = np.asarray(inp["W5"], np.float32).T             # [64, 1]
    b5 = np.asarray(inp["b5"], np.float32).reshape(1, 1)

    iota = np.broadcast_to(np.arange(128, dtype=np.float32), (128, 128)
                           ).astype(ml_dtypes.bfloat16)
    ident_bf = np.eye(128, dtype=ml_dtypes.bfloat16)

    shared = {
        "w1c": w1c,
        "w2l": np.asarray(inp["Wl2"], np.float32).T.astype(ml_dtypes.bfloat16),
        "w2r": np.asarray(inp["Wr2"], np.float32).T.astype(ml_dtypes.bfloat16),
        "w3l": np.asarray(inp["Wl3"], np.float32).T.astype(ml_dtypes.bfloat16),
        "w3r": np.asarray(inp["Wr3"], np.float32).T.astype(ml_dtypes.bfloat16),
        "sc1": sc1, "bs1": bs1, "sc2": sc2, "bs2": bs2, "sc3": sc3, "bs3": bs3,
        "w4h": w4h, "w4x": w4x, "b4": b4, "w5": w5, "b5": b5,
        "iota": iota, "ident_bf": ident_bf,
    }

    eidx = [np.asarray(inp[f"edge_index_{k}"]) for k in range(3)]
    ea0 = np.asarray(inp["edge_attr_0"], np.float32)
    sched = compute_sched(cfg, eidx)

    in_maps = []
    for c in range(cfg.NC):
        m = dict(shared)
        # x_ownT bf16 [D_IN, NPAD]
        xo = np.zeros((cfg.NPAD, D_IN), np.float32)
        lo, hi = c * cfg.NSH, min((c + 1) * cfg.NSH, cfg.N)
        xo[:hi - lo] = x[lo:hi]
        m["x_ownT"] = xo.T.astype(ml_dtypes.bfloat16).copy()
        for k in range(3):
            p = prep_core_hop(cfg, sched[k], eidx[k][0], eidx[k][1], c,
                              x=x if k == 0 else None,
                              edge_attr=ea0 if k == 0 else None)
            m[f"dstrel{k}"] = p["dstrel"]
            m[f"inv{k}"] = p["inv"]
            if k == 0:
                m["gx0"] = p["gx"]
            else:
                m[f"gidx{k}"] = p["gidx"]
                m[f"cnt{k}"] = p["cnt"]
        in_maps.append(m)
    return in_maps, sched


def build_kernel(cfg: Cfg, sched, queue_map=None):
    gather_insts = {}  # inst name -> key

    nc = bacc.Bacc("TRN2", target_bir_lowering=False, debug=False,
                   num_devices=cfg.NC, num_swdge_queues=4,
                   dynamic_dma_scratch_size=16384)
    H, D_IN, D_E = cfg.H, cfg.D_IN, cfg.D_E
    DXE = D_IN + D_E
    NW = cfg.NW
    WPB = 2 if NW % 2 == 0 else 1
    NBLK = NW // WPB
    lay = [sched_layout(cfg, sub) for sub in sched]
    MAXSUBA0 = int(sched[0][0].max())
    MAXSUBA = max(int(sub[0].max()) for sub in sched[1:])
    MAXSUBB = max(max(int(sub[1].max()), 1) for sub in sched[1:])
    MA = max(WPB * MAXSUBA, MAXSUBA0)
    MB = WPB * MAXSUBB

    P = {}

    def par(name, shape, dt=F32, out=False):
        P[name] = nc.declare_dram_parameter(name, list(shape), dt, isOutput=out)
        return P[name]

    par("gx0", (128, lay[0][5] // 128, 128), BF16)
    par("x_ownT", (D_IN, cfg.NPAD), BF16)
    for k in range(3):
        par(f"dstrel{k}", (128, lay[k][5] // 128), BF16)
        par(f"inv{k}", (128, cfg.NPAD), BF16)
        if k > 0:
            par(f"gidx{k}", (128, lay[k][5] // 16), I16)
            par(f"cnt{k}", (1, 128), I32)
    par("w1c", (DXE + 1, H), BF16)
    par("w2l", (H, H), BF16); par("w2r", (H, H), BF16)
    par("w3l", (H, H), BF16); par("w3r", (H, H), BF16)
    for nm in ("sc1", "bs1", "sc2", "bs2", "sc3", "bs3"):
        par(nm, (H, 1))
    par("w4h", (H, 64), BF16); par("w4x", (D_IN, 64), BF16)
    par("b4", (64, 1)); par("w5", (64, 1)); par("b5", (1, 1))
    par("iota", (128, 128), BF16)
    par("ident_bf", (128, 128), BF16)
    out_ext = par("out", (1, cfg.NPAD), out=True)

    with tile.TileContext(nc) as tc:
        with (
            tc.tile_pool(name="const", bufs=1) as cp,
            tc.tile_pool(name="invp", bufs=2) as invp,
            tc.tile_pool(name="cnp", bufs=3) as cnp,
            tc.tile_pool(name="ohp", bufs=4) as ohp,
            tc.tile_pool(name="ip", bufs=4) as ip,
            tc.tile_pool(name="hp", bufs=2) as hp,
            tc.tile_pool(name="nmp", bufs=6) as nmp,
            tc.tile_pool(name="pse", bufs=3, space="PSUM") as pse,
            tc.tile_pool(name="psn", bufs=5, space="PSUM") as psn,
            tc.tile_pool(name="dram", bufs=1, space="DRAM") as dp,
        ):
            def ld(name, dt=F32):
                t = cp.tile(list(P[name].shape), dt, tag=name)
                nc.scalar.dma_start(t[:], P[name].ap())
                return t

            w1c = ld("w1c", BF16)
            w2l = ld("w2l", BF16); w3l = ld("w3l", BF16)
            w2r = ld("w2r", BF16); w3r = ld("w3r", BF16)
            sc = [ld(f"sc{k}") for k in (1, 2, 3)]
            bs = [ld(f"bs{k}") for k in (1, 2, 3)]
            w4h = ld("w4h", BF16); w4x = ld("w4x", BF16)
            b4 = ld("b4"); w5 = ld("w5"); b5 = ld("b5")
            iota = ld("iota", BF16)
            ident_bf = ld("ident_bf", BF16)
            x_ownT = ld("x_ownT", BF16)
            iota_big = cp.tile([128, MA, 128], BF16, tag="iota_big")
            nc.vector.tensor_copy(
                iota_big[:],
                iota[:].rearrange("p (o f) -> p o f", o=1).broadcast_to(
                    [128, MA, 128]))

            # persistent gather ring buffers: pad slots skip their DMA and
            # must read as finite values for the masked matmul, so the rings
            # are zeroed once here and then only ever overwritten by gathers
            # (or hop-0's streamed blocks, which cover every slot they read).
            RING_A = 6
            RING_B = 6
            gbufA = cp.tile([128, RING_A * MA, 128], BF16, tag="gbufA")
            gbufB = cp.tile([128, RING_B * MB, 128], BF16, tag="gbufB")
            for rp in range(RING_A):
                nc.vector.memset(gbufA[:, rp * MA:(rp + 1) * MA, :], 0.0)
            for rp in range(RING_B):
                nc.vector.memset(gbufB[:, rp * MB:(rp + 1) * MB, :], 0.0)

            htblX = [dp.tile([cfg.NC * cfg.XROWS, 128], BF16,
                             name=f"htblX{k}", tag=f"htblX{k}",
                             addr_space="Shared") for k in range(2)]
            htblY = [dp.tile([cfg.NC * cfg.YROWS, 128], BF16,
                             name=f"htblY{k}", tag=f"htblY{k}",
                             addr_space="Shared") for k in range(2)]
            bounceX = [dp.tile([cfg.XROWS, 128], BF16, name=f"bounceX{k}",
                               tag=f"bounceX{k}") for k in range(2)]
            bounceY = [dp.tile([cfg.YROWS, 128], BF16, name=f"bounceY{k}",
                               tag=f"bounceY{k}") for k in range(2)]

            # tiny warm-up collective: absorbs first-call AG overhead
            wub = dp.tile([128, 128], BF16, tag="wub")
            wuo = dp.tile([cfg.NC * 128, 128], BF16, tag="wuo",
                          addr_space="Shared")
            wz = cp.tile([128, 128], BF16, tag="wz")
            nc.vector.memset(wz[:], 0.0)
            nc.sync.dma_start(wub[:], wz[:])
            nc.gpsimd.collective_compute(
                "AllGather", ALU.bypass,
                replica_groups=[list(range(cfg.NC))],
                ins=[wub.opt()], outs=[wuo.opt()])

            h_prev = None
            h_cur = None
            cnt_regs = [nc.gpsimd.alloc_register(f"cntreg{i}")
                        for i in range(12)]
            creg_ctr = [0]
            ring_ctr = {"A": 0, "B": 0}

            def load_cnt(ap):
                r = cnt_regs[creg_ctr[0] % 12]
                creg_ctr[0] += 1
                nc.gpsimd.reg_load(r, ap)
                return r

            def reg_gather(inst, key):
                gather_insts[inst.ins.name] = key

            for k in range(3):
                baseA, baseB, slotsA, slotsB, a_tot, tot = lay[k]
                subsA, subsB = sched[k]
                WPBk = 1 if k == 0 else WPB
                NBLKk = NW // WPBk
                PRO = 4 if k else 3   # gather/stream lookahead depth (blocks)
                if k > 0:
                    tblX = htblX[k - 1][:]
                    tblY = htblY[k - 1][:]
                inv = invp.tile([128, cfg.NPAD], BF16, tag="inv_rep",
                                name=f"invt{k}")
                nc.scalar.dma_start(inv[:], P[f"inv{k}"].ap())
                if k > 0:
                    cntT = cnp.tile([1, 128], I32, tag="cntT", name=f"cntT{k}")
                    nc.sync.dma_start(cntT[:], P[f"cnt{k}"].ap())

                h_prev = h_cur
                h_cur = hp.tile([128, cfg.NPAD], BF16, tag="h", name=f"h{k}")
                if k < 2:
                    bsb = cp.tile([128, cfg.NWR, 128], BF16,
                                  tag="bsb", name=f"bsb{k}")
                fdim = DXE + 1 if k == 0 else 128
                state = {}

                def emit_a(j, k=k, state=state,
                           subsA=subsA, subsB=subsB, baseA=baseA, baseB=baseB,
                           WPBk=WPBk):
                    ws = list(range(j * WPBk, (j + 1) * WPBk))
                    nsa = [int(subsA[w]) for w in ws]
                    nsb = [int(subsB[w]) for w in ws]
                    nA = 128 * sum(nsa)
                    nB = 128 * sum(nsb)
                    sA0 = int(baseA[ws[0]])
                    sB0 = int(baseB[ws[0]])
                    rp = ring_ctr["A"] % RING_A
                    ring_ctr["A"] += 1
                    ga = gbufA[:, rp * MA:rp * MA + sum(nsa), :]
                    if k == 0:
                        nc.sync.dma_start(
                            ga[:], P["gx0"].ap()[:, sA0 // 128:
                                                 (sA0 + nA) // 128, :])
                        state[j] = (ga, None, nsa, nsb, nA, nB, sA0, sB0)
                        return
                    gia = ip.tile([128, nA // 16], I16, tag="gia",
                                  name=f"gia{k}_{j}")
                    nc.sync.dma_start(
                        gia[:], P[f"gidx{k}"].ap()[:, sA0 // 16:
                                                   (sA0 + nA) // 16])
                    for wl in range(WPBk):
                        w = ws[wl]
                        offa = sum(nsa[:wl])
                        na_w = nsa[wl] * 128
                        cva = load_cnt(cntT[0:1, w:w + 1])
                        keyA = (k, j, wl, 0)
                        reg_gather(nc.gpsimd.dma_gather(
                            ga[:, offa:offa + nsa[wl], :], tblX,
                            gia[:, (offa * 128) // 16:
                                (offa * 128 + na_w) // 16],
                            na_w, cva, 128, single_packet=False,
                            queue_num=(queue_map or {}).get(keyA, 0)), keyA)
                    state[j] = (ga, None, nsa, nsb, nA, nB, sA0, sB0)

                def emit_b(j, k=k, state=state,
                           subsB=subsB, baseB=baseB, WPBk=WPBk):
                    if k == 0:
                        return
                    ga, _, nsa, nsb, nA, nB, sA0, sB0 = state[j]
                    ws = list(range(j * WPBk, (j + 1) * WPBk))
                    if nB == 0:
                        return
                    rp = ring_ctr["B"] % RING_B
                    ring_ctr["B"] += 1
                    gb = gbufB[:, rp * MB:rp * MB + max(sum(nsb), 1), :]
                    gib = ip.tile([128, nB // 16], I16, tag="gib",
                                  name=f"gib{k}_{j}")
                    nc.sync.dma_start(
                        gib[:], P[f"gidx{k}"].ap()[:, sB0 // 16:
                                                   (sB0 + nB) // 16])
                    for wl in range(WPBk):
                        w = ws[wl]
                        if nsb[wl] > 0:
                            offb = sum(nsb[:wl])
                            nb_w = nsb[wl] * 128
                            cvb = load_cnt(cntT[0:1, 64 + w:64 + w + 1])
                            keyB = (k, j, wl, 1)
                            reg_gather(nc.gpsimd.dma_gather(
                                gb[:, offb:offb + nsb[wl], :], tblY,
                                gib[:, (offb * 128) // 16:
                                    (offb * 128 + nb_w) // 16],
                                nb_w, cvb, 128, single_packet=False,
                                queue_num=(queue_map or {}).get(keyB, 0)), keyB)
                    state[j] = (ga, gb, nsa, nsb, nA, nB, sA0, sB0)

                def emit_rest(j, k=k, state=state, inv=inv, h_cur=h_cur,
                              h_prev=h_prev, fdim=fdim, WPBk=WPBk,
                              bsb=(bsb if k < 2 else None)):
                    ga, gb, nsa, nsb, nA, nB, sA0, sB0 = state.pop(j)
                    ws = list(range(j * WPBk, (j + 1) * WPBk))
                    oa = ohp.tile([128, sum(nsa), 128], BF16, tag="oa",
                                  name=f"oa{k}_{j}")
                    dra = ip.tile([128, sum(nsa)], BF16, tag="dra",
                                  name=f"dra{k}_{j}")
                    nc.sync.dma_start(
                        dra[:], P[f"dstrel{k}"].ap()[:, sA0 // 128:
                                                     (sA0 + nA) // 128])
                    nc.vector.tensor_tensor(
                        oa[:], iota_big[:, 0:sum(nsa), :],
                        dra[:].rearrange("p (s o) -> p s o", o=1).broadcast_to(
                            [128, sum(nsa), 128]),
                        ALU.is_equal)
                    ob = None
                    if nB > 0:
                        ob = ohp.tile([128, sum(nsb), 128], BF16, tag="ob",
                                      name=f"ob{k}_{j}")
                        drb = ip.tile([128, sum(nsb)], BF16, tag="drb",
                                      name=f"drb{k}_{j}")
                        nc.sync.dma_start(
                            drb[:], P[f"dstrel{k}"].ap()[:, sB0 // 128:
                                                         (sB0 + nB) // 128])
                        nc.vector.tensor_tensor(
                            ob[:], iota_big[:, 0:sum(nsb), :],
                            drb[:].rearrange(
                                "p (s o) -> p s o", o=1).broadcast_to(
                                [128, sum(nsb), 128]),
                            ALU.is_equal)

                    for wl in range(WPBk):
                        w = ws[wl]
                        offa = sum(nsa[:wl])
                        offb = sum(nsb[:wl])
                        cols = slice(w * 128, (w + 1) * 128)
                        nmm = nsa[wl] + nsb[wl]
                        mi = 0
                        ps = pse.tile([128, 128], F32, tag="ps",
                                      name=f"ps{k}_{w}")
                        for t in range(nsa[wl]):
                            nc.tensor.matmul(
                                ps[0:fdim, :], ga[:, offa + t, 0:fdim],
                                oa[:, offa + t, :],
                                start=(mi == 0), stop=(mi == nmm - 1))
                            mi += 1
                        for t in range(nsb[wl]):
                            nc.tensor.matmul(
                                ps[0:fdim, :], gb[:, offb + t, 0:fdim],
                                ob[:, offb + t, :],
                                start=(mi == 0), stop=(mi == nmm - 1))
                            mi += 1
                        rhs = nmp.tile([128, 128], BF16, tag="rhs",
                                       name=f"rhs{k}_{w}")
                        nc.vector.tensor_tensor(rhs[0:fdim, :], ps[0:fdim, :],
                                                inv[0:fdim, cols], ALU.mult)
                        ps2 = psn.tile([128, 128], F32, tag="psn",
                                       name=f"ps2{k}_{w}")
                        if k == 0:
                            nc.tensor.matmul(ps2[:], w1c[:], rhs[0:fdim, :],
                                             start=True, stop=True)
                            tmp = nmp.tile([128, 128], F32, tag="tmp",
                                           name=f"tmp{w}")
                            nc.scalar.activation(tmp[:], ps2[:], AF.Relu)
                            nc.scalar.activation(h_cur[:, cols], tmp[:],
                                                 AF.Relu, bias=bs[0][:],
                                                 scale=sc[0][:])
                        else:
                            wl_ = w2l if k == 1 else w3l
                            wr_ = w2r if k == 1 else w3r
                            nc.tensor.matmul(ps2[:], wl_[:], rhs[:],
                                             start=True, stop=False)
                            nc.tensor.matmul(ps2[:], wr_[:], h_prev[:, cols],
                                             start=False, stop=True)
                            nc.scalar.activation(h_cur[:, cols], ps2[:],
                                                 AF.Relu, bias=bs[k][:],
                                                 scale=sc[k][:])
                        if k < 2 and w < cfg.NWR:
                            pstr = psn.tile([128, 128], BF16, tag="psn",
                                            name=f"pstr{k}_{w}")
                            nc.tensor.transpose(pstr[:], h_cur[:, cols],
                                                ident_bf[:])
                            nc.scalar.activation(bsb[:, w, :], pstr[:],
                                                 AF.Copy)
                            XW = cfg.XW
                            NWR = cfg.NWR
                            if w < XW and (w % 8 == 7 or w == XW - 1):
                                w0 = (w // 8) * 8
                                nc.sync.dma_start(
                                    bounceX[k][:].rearrange(
                                        "(t p) f -> p t f",
                                        p=128)[:, w0:w + 1, :],
                                    bsb[:, w0:w + 1, :])
                                if w == XW - 1:
                                    nc.gpsimd.collective_compute(
                                        "AllGather", ALU.bypass,
                                        replica_groups=[list(range(cfg.NC))],
                                        ins=[bounceX[k].opt()],
                                        outs=[htblX[k].opt()])
                            elif w >= XW and ((w - XW) % 8 == 7
                                              or w == NWR - 1):
                                w0 = XW + ((w - XW) // 8) * 8
                                nc.sync.dma_start(
                                    bounceY[k][:].rearrange(
                                        "(t p) f -> p t f",
                                        p=128)[:, w0 - XW:w + 1 - XW, :],
                                    bsb[:, w0:w + 1, :])
                                if w == NWR - 1:
                                    nc.gpsimd.collective_compute(
                                        "AllGather", ALU.bypass,
                                        replica_groups=[list(range(cfg.NC))],
                                        ins=[bounceY[k].opt()],
                                        outs=[htblY[k].opt()])
                        if k == 2:
                            ps4 = psn.tile([128, 128], F32, tag="psn",
                                           name=f"ps4_{w}")
                            nc.tensor.matmul(ps4[0:64, :], w4h[:],
                                             h_cur[:, cols],
                                             start=True, stop=False)
                            nc.tensor.matmul(ps4[0:64, :], w4x[:],
                                             x_ownT[:, cols],
                                             start=False, stop=True)
                            z = nmp.tile([64, 128], F32, tag="z",
                                         name=f"z{w}")
                            nc.scalar.activation(z[:], ps4[0:64, :], AF.Relu,
                                                 bias=b4[:])
                            ps5 = psn.tile([128, 128], F32, tag="psn",
                                           name=f"ps5_{w}")
                            nc.tensor.matmul(ps5[0:1, :], w5[:], z[:],
                                             start=True, stop=True)
                            z5 = nmp.tile([1, 128], F32, tag="z5",
                                          name=f"z5_{w}")
                            nc.scalar.activation(z5[:], ps5[0:1, :],
                                                 AF.Identity,
                                                 bias=b5[0:1, :])
                            nc.sync.dma_start(out_ext.ap()[:, cols], z5[:])

                for j in range(NBLKk + PRO):
                    if j < NBLKk:
                        emit_a(j)
                        emit_b(j)
                    if j >= PRO:
                        emit_rest(j - PRO)

    nc.compile()
    return nc, gather_insts


def final_queue_map(nc, gather_insts):
    """Lane i (mod 8, final program order over Pool DMA insts) must keep a
    consistent SWDGE queue. Choose the lane->queue map to balance bytes."""
    lane_keys = [[] for _ in range(8)]
    lane_bytes = [0] * 8
    idx = 0
    for bb in nc.m.functions[0].blocks:
        for inst in bb.instructions:
            if type(inst).__name__ == "InstDMAGatherAnt":
                key = gather_insts.get(inst.name)
                assert key is not None, inst.name
                lane_keys[idx % 8].append(key)
                lane_bytes[idx % 8] += inst.num_idxs
                idx += 1
    # Strict round-robin: program-consecutive Pool DMA insts land on
    # different queues, so their Q7 descriptor generation overlaps.
    qmap = {}
    for l in range(8):
        for key in lane_keys[l]:
            qmap[key] = l % 4
    return qmap


def build_kernel2(cfg, sched):
    nc1, gi1 = build_kernel(cfg, sched)
    qmap = final_queue_map(nc1, gi1)
    nc2, _ = build_kernel(cfg, sched, queue_map=qmap)
    return nc2


def assemble_output(cfg: Cfg, results):
    out = np.zeros(cfg.N, np.float32)
    for c, r in enumerate(results):
        lo, hi = c * cfg.NSH, min((c + 1) * cfg.NSH, cfg.N)
        out[lo:hi] = np.asarray(r["out"], np.float32).reshape(-1)[:hi - lo]
    return out


# ======================================================================
# Self-contained entry point: kernel(**inputs) -> np.ndarray [N] float32
# ======================================================================
from concourse.bass_utils import run_bass_kernel_spmd

_BUILD_CACHE = {}


def _get_nc(cfg, sched):
    key = tuple((tuple(a), tuple(b)) for a, b in sched)
    nc = _BUILD_CACHE.get(key)
    if nc is None:
        nc = build_kernel2(cfg, sched)
        _BUILD_CACHE[key] = nc
    return nc


def kernel(**inputs):
    cfg = FULL
    inp = {k: np.asarray(v) for k, v in inputs.items()}
    in_maps, sched = prep_inputs(cfg, inp)
    nc = _get_nc(cfg, sched)
    res = run_bass_kernel_spmd(nc, in_maps, core_ids=list(range(cfg.NC)),
                               trace=False)
    return assemble_output(cfg, res.results)


# revision 10
# speedup vs baseline: 1.3771x; 1.0118x over previous
"""GNN message-passing (ArtemisNet) distributed Bass kernel for 8 TRN2 cores, v3.

Strategy (v3 — hop-0 host-gathered stream + descriptor-exact gathers):
- dst-sharding: core c owns nodes [c*NSH, (c+1)*NSH). Edges assigned by dst.
- Hop 0: the gather of x[src] is fully resolved on the host into a
  slot-major, partition-major stream gx0[p, t, :] = [x|ea|1] of slot t*128+p,
  zero-padded. The kernel streams it with plain (HWDGE) dma_start — no
  GpSimd descriptor generation, no separate edge-attr matmuls.
- Hops 1-2: gather of h rows via dma_gather (int16 idx, X/Y table halves,
  chunk-major table layout). Per-(window,half) gather calls with runtime
  exact counts (num_idxs_reg via value_load): pad slots emit NO descriptors.
- Segment aggregation on TensorEngine: per 128-dst window, PSUM accumulates
  G_sub^T @ onehot_sub; pad slots have dstrel=-1 -> zero one-hot columns.
- Node-wise GEMMs feature-major; BN+ReLU folded into one ACT op.
- h tables republished per hop via per-half AllGathers, so the next hop's
  X gathers wait only on the X-half publish.
"""

import dataclasses
import numpy as np
import ml_dtypes

import concourse.bass as bass
import concourse.bacc as bacc
import concourse.tile as tile
import concourse.mybir as mybir

BF16 = mybir.dt.bfloat16
F32 = mybir.dt.float32
I16 = mybir.dt.int16
I32 = mybir.dt.int32
AF = mybir.ActivationFunctionType
ALU = mybir.AluOpType


@dataclasses.dataclass
class Cfg:
    N: int = 50000
    E: int = 800000
    NC: int = 8
    D_IN: int = 64
    D_E: int = 32
    H: int = 128
    EPS: float = 1e-5
    NSH: int = 6250          # nodes per core
    WSZ: int = 128           # dst window size
    NW: int = 50             # windows per core (NW*WSZ >= NSH)
    SHPAD: int = 6272        # padded shard rows in gather table (mult of 128)

    @property
    def NPAD(self):
        return self.NW * self.WSZ

    @property
    def NWR(self):
        return self.SHPAD // 128

    @property
    def XROWS(self):
        return (self.SHPAD // 128 * 31 // 49) * 128 if self.SHPAD > 256 else self.SHPAD // 2

    @property
    def YROWS(self):
        return self.SHPAD - self.XROWS

    @property
    def XW(self):
        return self.XROWS // 128

    def tblrow(self, s):
        """Vectorized: global node id -> (half(0=X,1=Y), table row)."""
        s = np.asarray(s, np.int64)
        sh = s // self.NSH
        r = s % self.NSH
        half = (r >= self.XROWS).astype(np.int64)
        row = np.where(half == 0, sh * self.XROWS + r,
                       sh * self.YROWS + (r - self.XROWS))
        return half, row


FULL = Cfg()


def _wrap_idx16(a):
    """[n] int -> [128, n//16] int16 (idx i at partition i%16, col i//16; tiled x8)."""
    n = a.shape[0]
    assert n % 16 == 0
    w = a.reshape(n // 16, 16).T.astype(np.int16)
    return np.tile(w, (8, 1)).copy()


def _slotmaj(a):
    """[TOT] -> [128, TOT//128] slot i at [i%128, i//128]."""
    t = a.shape[0]
    return np.ascontiguousarray(a.reshape(t // 128, 128).T)


def hop_counts(cfg: Cfg, src, dst, c, halves=True):
    sel = (dst >= c * cfg.NSH) & (dst < (c + 1) * cfg.NSH)
    s = src[sel].astype(np.int64)
    d = (dst[sel] - c * cfg.NSH).astype(np.int64)
    half = cfg.tblrow(s)[0] if halves else np.zeros_like(s)
    win = d // cfg.WSZ
    key = half * cfg.NW + win
    return np.bincount(key, minlength=2 * cfg.NW)


def compute_sched(cfg: Cfg, eidx):
    """Per-hop per-window sub-chunk counts (max over cores), SPMD-static."""
    sched = []
    for k in range(3):
        mx = np.zeros(2 * cfg.NW, np.int64)
        for c in range(cfg.NC):
            mx = np.maximum(mx, hop_counts(cfg, eidx[k][0], eidx[k][1], c,
                                           halves=(k > 0)))
        subsA = np.maximum(1, -(-mx[:cfg.NW] // 128))
        subsB = -(-mx[cfg.NW:] // 128)
        sched.append((subsA.astype(int), subsB.astype(int)))
    return sched


def sched_layout(cfg: Cfg, sub):
    """Slot bases per (half, window) from a hop schedule."""
    subsA, subsB = sub
    slotsA, slotsB = subsA * 128, subsB * 128
    a_tot = int(slotsA.sum())
    baseA = np.concatenate([[0], np.cumsum(slotsA)[:-1]])
    baseB = a_tot + np.concatenate([[0], np.cumsum(slotsB)[:-1]])
    tot = a_tot + int(slotsB.sum())
    return baseA, baseB, slotsA, slotsB, a_tot, tot


def prep_core_hop(cfg: Cfg, sub, src, dst, c, x=None, edge_attr=None):
    """Slot assignment for one (core, hop) under schedule `sub`.

    Pads are trailing within each (window, half) segment: gidx=-1, dstrel=-1.
    Returns per-(window,half) valid counts for runtime-exact gathers.

    Hop 0 (x is not None): all edges in the A "half"; instead of gather
    indices, emits the fully host-gathered slot stream gx[128, tot//128, 128].
    """
    hop0 = x is not None
    baseA, baseB, slotsA, slotsB, a_tot, tot = sched_layout(cfg, sub)
    sel = (dst >= c * cfg.NSH) & (dst < (c + 1) * cfg.NSH)
    s = src[sel].astype(np.int64)
    d = (dst[sel] - c * cfg.NSH).astype(np.int64)
    if hop0:
        half = np.zeros_like(s)
        tblrow = s  # unused
    else:
        half, tblrow = cfg.tblrow(s)
    win = d // cfg.WSZ

    dstrel = np.full(tot, -1.0, np.float32)

    order = np.lexsort((d, win, half))
    s_, d_, t_, h_, w_ = (v[order] for v in (s, d, tblrow, half, win))

    deg = np.bincount(d, minlength=cfg.NPAD).astype(np.float32)
    keys = h_ * cfg.NW + w_
    bnd = np.searchsorted(keys, np.arange(2 * cfg.NW + 1))
    cnts = (bnd[1:] - bnd[:-1]).astype(np.int64)
    slots_per = np.concatenate([slotsA, slotsB])
    assert (cnts <= slots_per).all(), f"slot overflow core {c}"
    seg_base = np.concatenate([baseA, baseB])
    pos = (seg_base[keys] + np.arange(len(keys)) - bnd[keys]).astype(np.int64)
    dstrel[pos] = (d_ - w_ * cfg.WSZ).astype(np.float32)

    invdeg = (1.0 / np.maximum(deg, 1.0)).astype(np.float32)
    out = {
        "inv": np.broadcast_to(invdeg.astype(ml_dtypes.bfloat16),
                               (128, cfg.NPAD)).copy(),
        "dstrel": _slotmaj(dstrel.astype(ml_dtypes.bfloat16)),
    }

    if hop0:
        gx = np.zeros((tot, 128), np.float32)
        gx[pos, :cfg.D_IN] = x[s_]
        gx[pos, cfg.D_IN:cfg.D_IN + cfg.D_E] = edge_attr[sel][order]
        gx[pos, cfg.D_IN + cfg.D_E] = 1.0
        out["gx"] = np.ascontiguousarray(
            gx.reshape(tot // 128, 128, 128).transpose(1, 0, 2)
        ).astype(ml_dtypes.bfloat16)
        return out

    gidx = np.full(tot, -1, np.int64)
    gidx[pos] = t_

    # zero-count segments with scheduled slots: keep one valid dummy desc
    # (row 0, dstrel=-1) so num_idxs_reg >= 1 everywhere.
    for seg in range(2 * cfg.NW):
        if cnts[seg] == 0 and slots_per[seg] > 0:
            gidx[seg_base[seg]] = 0
            cnts[seg] = 1

    cnt128 = np.zeros(128, np.int32)
    cnt128[:cfg.NW] = cnts[:cfg.NW]          # A counts
    cnt128[64:64 + cfg.NW] = cnts[cfg.NW:]   # B counts
    out["gidx"] = _wrap_idx16(gidx)
    out["cnt"] = cnt128.reshape(1, 128)
    return out


def prep_inputs(cfg: Cfg, inp):
    """Full-host preprocessing: returns in_maps (list of dicts, one per core)."""
    x = np.asarray(inp["x"], np.float32)
    H, D_IN, D_E = cfg.H, cfg.D_IN, cfg.D_E

    W1 = np.asarray(inp["W1"], np.float32)  # [H, D_IN+D_E]
    w1c = np.zeros((D_IN + D_E + 1, H), np.float32)
    w1c[:D_IN] = W1[:, :D_IN].T
    w1c[D_IN:D_IN + D_E] = W1[:, D_IN:].T
    w1c[D_IN + D_E] = np.asarray(inp["b1"], np.float32)
    w1c = w1c.astype(ml_dtypes.bfloat16)

    def bn_fold(g, be, m, v, blin=None):
        g, be, m, v = (np.asarray(inp[k], np.float32) for k in (g, be, m, v))
        gam = g / np.sqrt(v + cfg.EPS)
        bet = be - m * gam
        if blin is not None:
            bet = bet + gam * np.asarray(inp[blin], np.float32)
        return gam.reshape(-1, 1), bet.reshape(-1, 1)

    sc1, bs1 = bn_fold("g1", "be1", "m1", "v1")
    sc2, bs2 = bn_fold("g2", "be2", "m2", "v2", "bl2")
    sc3, bs3 = bn_fold("g3", "be3", "m3", "v3", "bl3")

    W4 = np.asarray(inp["W4"], np.float32)  # [64, H+D_IN]
    w4h = W4[:, :H].T.astype(ml_dtypes.bfloat16)         # [H, 64]
    w4x = W4[:, H:].T.astype(ml_dtypes.bfloat16)         # [D_IN, 64]
    b4 = np.asarray(inp["b4"], np.float32).reshape(-1, 1)
    w5 = np.asarray(inp["W5"], np.float32).T             # [64, 1]
    b5 = np.asarray(inp["b5"], np.float32).reshape(1, 1)

    iota = np.broadcast_to(np.arange(128, dtype=np.float32), (128, 128)
                           ).astype(ml_dtypes.bfloat16)
    ident_bf = np.eye(128, dtype=ml_dtypes.bfloat16)

    shared = {
        "w1c": w1c,
        "w2l": np.asarray(inp["Wl2"], np.float32).T.astype(ml_dtypes.bfloat16),
        "w2r": np.asarray(inp["Wr2"], np.float32).T.astype(ml_dtypes.bfloat16),
        "w3l": np.asarray(inp["Wl3"], np.float32).T.astype(ml_dtypes.bfloat16),
        "w3r": np.asarray(inp["Wr3"], np.float32).T.astype(ml_dtypes.bfloat16),
        "sc1": sc1, "bs1": bs1, "sc2": sc2, "bs2": bs2, "sc3": sc3, "bs3": bs3,
        "w4h": w4h, "w4x": w4x, "b4": b4, "w5": w5, "b5": b5,
        "iota": iota, "ident_bf": ident_bf,
    }

    eidx = [np.asarray(inp[f"edge_index_{k}"]) for k in range(3)]
    ea0 = np.asarray(inp["edge_attr_0"], np.float32)
    sched = compute_sched(cfg, eidx)

    in_maps = []
    for c in range(cfg.NC):
        m = dict(shared)
        # x_ownT bf16 [D_IN, NPAD]
        xo = np.zeros((cfg.NPAD, D_IN), np.float32)
        lo, hi = c * cfg.NSH, min((c + 1) * cfg.NSH, cfg.N)
        xo[:hi - lo] = x[lo:hi]
        m["x_ownT"] = xo.T.astype(ml_dtypes.bfloat16).copy()
        for k in range(3):
            p = prep_core_hop(cfg, sched[k], eidx[k][0], eidx[k][1], c,
                              x=x if k == 0 else None,
                              edge_attr=ea0 if k == 0 else None)
            m[f"dstrel{k}"] = p["dstrel"]
            m[f"inv{k}"] = p["inv"]
            if k == 0:
                m["gx0"] = p["gx"]
            else:
                m[f"gidx{k}"] = p["gidx"]
                m[f"cnt{k}"] = p["cnt"]
        in_maps.append(m)
    return in_maps, sched


def build_kernel(cfg: Cfg, sched, queue_map=None):
    gather_insts = {}  # inst name -> key

    nc = bacc.Bacc("TRN2", target_bir_lowering=False, debug=False,
                   num_devices=cfg.NC, num_swdge_queues=4,
                   dynamic_dma_scratch_size=16384)
    H, D_IN, D_E = cfg.H, cfg.D_IN, cfg.D_E
    DXE = D_IN + D_E
    NW = cfg.NW
    WPB = 2 if NW % 2 == 0 else 1
    NBLK = NW // WPB
    lay = [sched_layout(cfg, sub) for sub in sched]
    MAXSUBA0 = int(sched[0][0].max())
    MAXSUBA = max(int(sub[0].max()) for sub in sched[1:])
    MAXSUBB = max(max(int(sub[1].max()), 1) for sub in sched[1:])
    MA = max(WPB * MAXSUBA, MAXSUBA0)
    MB = WPB * MAXSUBB

    P = {}

    def par(name, shape, dt=F32, out=False):
        P[name] = nc.declare_dram_parameter(name, list(shape), dt, isOutput=out)
        return P[name]

    par("gx0", (128, lay[0][5] // 128, 128), BF16)
    par("x_ownT", (D_IN, cfg.NPAD), BF16)
    for k in range(3):
        par(f"dstrel{k}", (128, lay[k][5] // 128), BF16)
        par(f"inv{k}", (128, cfg.NPAD), BF16)
        if k > 0:
            par(f"gidx{k}", (128, lay[k][5] // 16), I16)
            par(f"cnt{k}", (1, 128), I32)
    par("w1c", (DXE + 1, H), BF16)
    par("w2l", (H, H), BF16); par("w2r", (H, H), BF16)
    par("w3l", (H, H), BF16); par("w3r", (H, H), BF16)
    for nm in ("sc1", "bs1", "sc2", "bs2", "sc3", "bs3"):
        par(nm, (H, 1))
    par("w4h", (H, 64), BF16); par("w4x", (D_IN, 64), BF16)
    par("b4", (64, 1)); par("w5", (64, 1)); par("b5", (1, 1))
    par("iota", (128, 128), BF16)
    par("ident_bf", (128, 128), BF16)
    out_ext = par("out", (1, cfg.NPAD), out=True)

    with tile.TileContext(nc) as tc:
        with (
            tc.tile_pool(name="const", bufs=1) as cp,
            tc.tile_pool(name="invp", bufs=2) as invp,
            tc.tile_pool(name="cnp", bufs=3) as cnp,
            tc.tile_pool(name="ohp", bufs=5) as ohp,
            tc.tile_pool(name="ip", bufs=4) as ip,
            tc.tile_pool(name="hp", bufs=2) as hp,
            tc.tile_pool(name="nmp", bufs=6) as nmp,
            tc.tile_pool(name="pse", bufs=3, space="PSUM") as pse,
            tc.tile_pool(name="psn", bufs=5, space="PSUM") as psn,
            tc.tile_pool(name="dram", bufs=1, space="DRAM") as dp,
        ):
            def ld(name, dt=F32):
                t = cp.tile(list(P[name].shape), dt, tag=name)
                nc.scalar.dma_start(t[:], P[name].ap())
                return t

            w1c = ld("w1c", BF16)
            w2l = ld("w2l", BF16); w3l = ld("w3l", BF16)
            w2r = ld("w2r", BF16); w3r = ld("w3r", BF16)
            sc = [ld(f"sc{k}") for k in (1, 2, 3)]
            bs = [ld(f"bs{k}") for k in (1, 2, 3)]
            w4h = ld("w4h", BF16); w4x = ld("w4x", BF16)
            b4 = ld("b4"); w5 = ld("w5"); b5 = ld("b5")
            iota = ld("iota", BF16)
            ident_bf = ld("ident_bf", BF16)
            x_ownT = ld("x_ownT", BF16)
            iota_big = cp.tile([128, MA, 128], BF16, tag="iota_big")
            nc.vector.tensor_copy(
                iota_big[:],
                iota[:].rearrange("p (o f) -> p o f", o=1).broadcast_to(
                    [128, MA, 128]))

            # persistent gather ring buffers: pad slots skip their DMA and
            # must read as finite values for the masked matmul, so the rings
            # are zeroed once here and then only ever overwritten by gathers
            # (or hop-0's streamed blocks, which cover every slot they read).
            RING_A = 6
            RING_B = 6
            gbufA = cp.tile([128, RING_A * MA, 128], BF16, tag="gbufA")
            gbufB = cp.tile([128, RING_B * MB, 128], BF16, tag="gbufB")
            for rp in range(RING_A):
                nc.vector.memset(gbufA[:, rp * MA:(rp + 1) * MA, :], 0.0)
            for rp in range(RING_B):
                nc.vector.memset(gbufB[:, rp * MB:(rp + 1) * MB, :], 0.0)

            htblX = [dp.tile([cfg.NC * cfg.XROWS, 128], BF16,
                             name=f"htblX{k}", tag=f"htblX{k}",
                             addr_space="Shared") for k in range(2)]
            htblY = [dp.tile([cfg.NC * cfg.YROWS, 128], BF16,
                             name=f"htblY{k}", tag=f"htblY{k}",
                             addr_space="Shared") for k in range(2)]
            bounceX = [dp.tile([cfg.XROWS, 128], BF16, name=f"bounceX{k}",
                               tag=f"bounceX{k}") for k in range(2)]
            bounceY = [dp.tile([cfg.YROWS, 128], BF16, name=f"bounceY{k}",
                               tag=f"bounceY{k}") for k in range(2)]

            # tiny warm-up collective: absorbs first-call AG overhead
            wub = dp.tile([128, 128], BF16, tag="wub")
            wuo = dp.tile([cfg.NC * 128, 128], BF16, tag="wuo",
                          addr_space="Shared")
            wz = cp.tile([128, 128], BF16, tag="wz")
            nc.vector.memset(wz[:], 0.0)
            nc.sync.dma_start(wub[:], wz[:])
            nc.gpsimd.collective_compute(
                "AllGather", ALU.bypass,
                replica_groups=[list(range(cfg.NC))],
                ins=[wub.opt()], outs=[wuo.opt()])

            h_prev = None
            h_cur = None
            cnt_regs = [nc.gpsimd.alloc_register(f"cntreg{i}")
                        for i in range(12)]
            creg_ctr = [0]
            ring_ctr = {"A": 0, "B": 0}

            def load_cnt(ap):
                r = cnt_regs[creg_ctr[0] % 12]
                creg_ctr[0] += 1
                nc.gpsimd.reg_load(r, ap)
                return r

            def reg_gather(inst, key):
                gather_insts[inst.ins.name] = key

            for k in range(3):
                baseA, baseB, slotsA, slotsB, a_tot, tot = lay[k]
                subsA, subsB = sched[k]
                WPBk = 1 if k == 0 else WPB
                NBLKk = NW // WPBk
                PRO = 4 if k else 3   # gather/stream lookahead depth (blocks)
                if k > 0:
                    tblX = htblX[k - 1][:]
                    tblY = htblY[k - 1][:]
                inv = invp.tile([128, cfg.NPAD], BF16, tag="inv_rep",
                                name=f"invt{k}")
                nc.scalar.dma_start(inv[:], P[f"inv{k}"].ap())
                if k > 0:
                    cntT = cnp.tile([1, 128], I32, tag="cntT", name=f"cntT{k}")
                    nc.sync.dma_start(cntT[:], P[f"cnt{k}"].ap())

                h_prev = h_cur
                h_cur = hp.tile([128, cfg.NPAD], BF16, tag="h", name=f"h{k}")
                if k < 2:
                    bsb = cp.tile([128, cfg.NWR, 128], BF16,
                                  tag="bsb", name=f"bsb{k}")
                fdim = DXE + 1 if k == 0 else 128
                state = {}

                def emit_a(j, k=k, state=state,
                           subsA=subsA, subsB=subsB, baseA=baseA, baseB=baseB,
                           WPBk=WPBk):
                    ws = list(range(j * WPBk, (j + 1) * WPBk))
                    nsa = [int(subsA[w]) for w in ws]
                    nsb = [int(subsB[w]) for w in ws]
                    nA = 128 * sum(nsa)
                    nB = 128 * sum(nsb)
                    sA0 = int(baseA[ws[0]])
                    sB0 = int(baseB[ws[0]])
                    rp = ring_ctr["A"] % RING_A
                    ring_ctr["A"] += 1
                    ga = gbufA[:, rp * MA:rp * MA + sum(nsa), :]
                    if k == 0:
                        nc.sync.dma_start(
                            ga[:], P["gx0"].ap()[:, sA0 // 128:
                                                 (sA0 + nA) // 128, :])
                        state[j] = (ga, None, nsa, nsb, nA, nB, sA0, sB0)
                        return
                    gia = ip.tile([128, nA // 16], I16, tag="gia",
                                  name=f"gia{k}_{j}")
                    nc.sync.dma_start(
                        gia[:], P[f"gidx{k}"].ap()[:, sA0 // 16:
                                                   (sA0 + nA) // 16])
                    for wl in range(WPBk):
                        w = ws[wl]
                        offa = sum(nsa[:wl])
                        na_w = nsa[wl] * 128
                        cva = load_cnt(cntT[0:1, w:w + 1])
                        keyA = (k, j, wl, 0)
                        reg_gather(nc.gpsimd.dma_gather(
                            ga[:, offa:offa + nsa[wl], :], tblX,
                            gia[:, (offa * 128) // 16:
                                (offa * 128 + na_w) // 16],
                            na_w, cva, 128, single_packet=False,
                            queue_num=(queue_map or {}).get(keyA, 0)), keyA)
                    state[j] = (ga, None, nsa, nsb, nA, nB, sA0, sB0)

                def emit_b(j, k=k, state=state,
                           subsB=subsB, baseB=baseB, WPBk=WPBk):
                    if k == 0:
                        return
                    ga, _, nsa, nsb, nA, nB, sA0, sB0 = state[j]
                    ws = list(range(j * WPBk, (j + 1) * WPBk))
                    if nB == 0:
                        return
                    rp = ring_ctr["B"] % RING_B
                    ring_ctr["B"] += 1
                    gb = gbufB[:, rp * MB:rp * MB + max(sum(nsb), 1), :]
                    gib = ip.tile([128, nB // 16], I16, tag="gib",
                                  name=f"gib{k}_{j}")
                    nc.sync.dma_start(
                        gib[:], P[f"gidx{k}"].ap()[:, sB0 // 16:
                                                   (sB0 + nB) // 16])
                    for wl in range(WPBk):
                        w = ws[wl]
                        if nsb[wl] > 0:
                            offb = sum(nsb[:wl])
                            nb_w = nsb[wl] * 128
                            cvb = load_cnt(cntT[0:1, 64 + w:64 + w + 1])
                            keyB = (k, j, wl, 1)
                            reg_gather(nc.gpsimd.dma_gather(
                                gb[:, offb:offb + nsb[wl], :], tblY,
                                gib[:, (offb * 128) // 16:
                                    (offb * 128 + nb_w) // 16],
                                nb_w, cvb, 128, single_packet=False,
                                queue_num=(queue_map or {}).get(keyB, 0)), keyB)
                    state[j] = (ga, gb, nsa, nsb, nA, nB, sA0, sB0)

                def emit_oh(j, k=k, state=state):
                    # One-hot build emitted right after the gathers, PRO
                    # blocks ahead of consumption: keeps the in-order DVE
                    # queue from convoying IS_EQ behind the inv-MULT (whose
                    # deps resolve PRO blocks later).
                    ga, gb, nsa, nsb, nA, nB, sA0, sB0 = state[j]
                    oa = ohp.tile([128, sum(nsa), 128], BF16, tag="oa",
                                  name=f"oa{k}_{j}")
                    dra = ip.tile([128, sum(nsa)], BF16, tag="dra",
                                  name=f"dra{k}_{j}")
                    nc.sync.dma_start(
                        dra[:], P[f"dstrel{k}"].ap()[:, sA0 // 128:
                                                     (sA0 + nA) // 128])
                    nc.vector.tensor_tensor(
                        oa[:], iota_big[:, 0:sum(nsa), :],
                        dra[:].rearrange("p (s o) -> p s o", o=1).broadcast_to(
                            [128, sum(nsa), 128]),
                        ALU.is_equal)
                    ob = None
                    if nB > 0:
                        ob = ohp.tile([128, sum(nsb), 128], BF16, tag="ob",
                                      name=f"ob{k}_{j}")
                        drb = ip.tile([128, sum(nsb)], BF16, tag="drb",
                                      name=f"drb{k}_{j}")
                        nc.sync.dma_start(
                            drb[:], P[f"dstrel{k}"].ap()[:, sB0 // 128:
                                                         (sB0 + nB) // 128])
                        nc.vector.tensor_tensor(
                            ob[:], iota_big[:, 0:sum(nsb), :],
                            drb[:].rearrange(
                                "p (s o) -> p s o", o=1).broadcast_to(
                                [128, sum(nsb), 128]),
                            ALU.is_equal)
                    state[j] = (ga, gb, nsa, nsb, nA, nB, sA0, sB0, oa, ob)

                def emit_rest(j, k=k, state=state, inv=inv, h_cur=h_cur,
                              h_prev=h_prev, fdim=fdim, WPBk=WPBk,
                              bsb=(bsb if k < 2 else None)):
                    ga, gb, nsa, nsb, nA, nB, sA0, sB0, oa, ob = state.pop(j)
                    ws = list(range(j * WPBk, (j + 1) * WPBk))

                    for wl in range(WPBk):
                        w = ws[wl]
                        offa = sum(nsa[:wl])
                        offb = sum(nsb[:wl])
                        cols = slice(w * 128, (w + 1) * 128)
                        nmm = nsa[wl] + nsb[wl]
                        mi = 0
                        ps = pse.tile([128, 128], F32, tag="ps",
                                      name=f"ps{k}_{w}")
                        for t in range(nsa[wl]):
                            nc.tensor.matmul(
                                ps[0:fdim, :], ga[:, offa + t, 0:fdim],
                                oa[:, offa + t, :],
                                start=(mi == 0), stop=(mi == nmm - 1))
                            mi += 1
                        for t in range(nsb[wl]):
                            nc.tensor.matmul(
                                ps[0:fdim, :], gb[:, offb + t, 0:fdim],
                                ob[:, offb + t, :],
                                start=(mi == 0), stop=(mi == nmm - 1))
                            mi += 1
                        rhs = nmp.tile([128, 128], BF16, tag="rhs",
                                       name=f"rhs{k}_{w}")
                        nc.vector.tensor_tensor(rhs[0:fdim, :], ps[0:fdim, :],
                                                inv[0:fdim, cols], ALU.mult)
                        ps2 = psn.tile([128, 128], F32, tag="psn",
                                       name=f"ps2{k}_{w}")
                        if k == 0:
                            nc.tensor.matmul(ps2[:], w1c[:], rhs[0:fdim, :],
                                             start=True, stop=True)
                            tmp = nmp.tile([128, 128], F32, tag="tmp",
                                           name=f"tmp{w}")
                            nc.scalar.activation(tmp[:], ps2[:], AF.Relu)
                            nc.scalar.activation(h_cur[:, cols], tmp[:],
                                                 AF.Relu, bias=bs[0][:],
                                                 scale=sc[0][:])
                        else:
                            wl_ = w2l if k == 1 else w3l
                            wr_ = w2r if k == 1 else w3r
                            nc.tensor.matmul(ps2[:], wl_[:], rhs[:],
                                             start=True, stop=False)
                            nc.tensor.matmul(ps2[:], wr_[:], h_prev[:, cols],
                                             start=False, stop=True)
                            nc.scalar.activation(h_cur[:, cols], ps2[:],
                                                 AF.Relu, bias=bs[k][:],
                                                 scale=sc[k][:])
                        if k < 2 and w < cfg.NWR:
                            pstr = psn.tile([128, 128], BF16, tag="psn",
                                            name=f"pstr{k}_{w}")
                            nc.tensor.transpose(pstr[:], h_cur[:, cols],
                                                ident_bf[:])
                            nc.scalar.activation(bsb[:, w, :], pstr[:],
                                                 AF.Copy)
                            XW = cfg.XW
                            NWR = cfg.NWR
                            if w < XW and (w % 8 == 7 or w == XW - 1):
                                w0 = (w // 8) * 8
                                nc.sync.dma_start(
                                    bounceX[k][:].rearrange(
                                        "(t p) f -> p t f",
                                        p=128)[:, w0:w + 1, :],
                                    bsb[:, w0:w + 1, :])
                                if w == XW - 1:
                                    nc.gpsimd.collective_compute(
                                        "AllGather", ALU.bypass,
                                        replica_groups=[list(range(cfg.NC))],
                                        ins=[bounceX[k].opt()],
                                        outs=[htblX[k].opt()])
                            elif w >= XW and ((w - XW) % 8 == 7
                                              or w == NWR - 1):
                                w0 = XW + ((w - XW) // 8) * 8
                                nc.sync.dma_start(
                                    bounceY[k][:].rearrange(
                                        "(t p) f -> p t f",
                                        p=128)[:, w0 - XW:w + 1 - XW, :],
                                    bsb[:, w0:w + 1, :])
                                if w == NWR - 1:
                                    nc.gpsimd.collective_compute(
                                        "AllGather", ALU.bypass,
                                        replica_groups=[list(range(cfg.NC))],
                                        ins=[bounceY[k].opt()],
                                        outs=[htblY[k].opt()])
                        if k == 2:
                            ps4 = psn.tile([128, 128], F32, tag="psn",
                                           name=f"ps4_{w}")
                            nc.tensor.matmul(ps4[0:64, :], w4h[:],
                                             h_cur[:, cols],
                                             start=True, stop=False)
                            nc.tensor.matmul(ps4[0:64, :], w4x[:],
                                             x_ownT[:, cols],
                                             start=False, stop=True)
                            z = nmp.tile([64, 128], F32, tag="z",
                                         name=f"z{w}")
                            nc.scalar.activation(z[:], ps4[0:64, :], AF.Relu,
                                                 bias=b4[:])
                            ps5 = psn.tile([128, 128], F32, tag="psn",
                                           name=f"ps5_{w}")
                            nc.tensor.matmul(ps5[0:1, :], w5[:], z[:],
                                             start=True, stop=True)
                            z5 = nmp.tile([1, 128], F32, tag="z5",
                                          name=f"z5_{w}")
                            nc.scalar.activation(z5[:], ps5[0:1, :],
                                                 AF.Identity,
                                                 bias=b5[0:1, :])
                            nc.sync.dma_start(out_ext.ap()[:, cols], z5[:])

                for j in range(NBLKk + PRO):
                    if j < NBLKk:
                        emit_a(j)
                        emit_b(j)
                        emit_oh(j)
                    if j >= PRO:
                        emit_rest(j - PRO)

    nc.compile()
    return nc, gather_insts


def final_queue_map(nc, gather_insts):
    """Lane i (mod 8, final program order over Pool DMA insts) must keep a
    consistent SWDGE queue. Choose the lane->queue map to balance bytes."""
    lane_keys = [[] for _ in range(8)]
    lane_bytes = [0] * 8
    idx = 0
    for bb in nc.m.functions[0].blocks:
        for inst in bb.instructions:
            if type(inst).__name__ == "InstDMAGatherAnt":
                key = gather_insts.get(inst.name)
                assert key is not None, inst.name
                lane_keys[idx % 8].append(key)
                lane_bytes[idx % 8] += inst.num_idxs
                idx += 1
    # Program-consecutive Pool DMA insts land on different queues so their
    # Q7 descriptor generation overlaps; the 4-7 lane group is rotated by 2
    # so the (heavier) A gathers alternate over all 4 queues across blocks
    # instead of pinning to queues 0-1.
    lane_to_q = {0: 0, 1: 1, 2: 2, 3: 3, 4: 2, 5: 3, 6: 0, 7: 1}
    qmap = {}
    for l in range(8):
        for key in lane_keys[l]:
            qmap[key] = lane_to_q[l]
    return qmap


def build_kernel2(cfg, sched):
    nc1, gi1 = build_kernel(cfg, sched)
    qmap = final_queue_map(nc1, gi1)
    nc2, _ = build_kernel(cfg, sched, queue_map=qmap)
    return nc2


def assemble_output(cfg: Cfg, results):
    out = np.zeros(cfg.N, np.float32)
    for c, r in enumerate(results):
        lo, hi = c * cfg.NSH, min((c + 1) * cfg.NSH, cfg.N)
        out[lo:hi] = np.asarray(r["out"], np.float32).reshape(-1)[:hi - lo]
    return out


# ======================================================================
# Self-contained entry point: kernel(**inputs) -> np.ndarray [N] float32
# ======================================================================
from concourse.bass_utils import run_bass_kernel_spmd

_BUILD_CACHE = {}


def _get_nc(cfg, sched):
    key = tuple((tuple(a), tuple(b)) for a, b in sched)
    nc = _BUILD_CACHE.get(key)
    if nc is None:
        nc = build_kernel2(cfg, sched)
        _BUILD_CACHE[key] = nc
    return nc


def kernel(**inputs):
    cfg = FULL
    inp = {k: np.asarray(v) for k, v in inputs.items()}
    in_maps, sched = prep_inputs(cfg, inp)
    nc = _get_nc(cfg, sched)
    res = run_bass_kernel_spmd(nc, in_maps, core_ids=list(range(cfg.NC)),
                               trace=False)
    return assemble_output(cfg, res.results)
